# revision 14
# baseline (speedup 1.0000x reference)
"""AttnBlock (GroupNorm + single-head self-attention + residual) on 8 Trainium2
NeuronCores, pure data-parallel over the batch dimension.

Reference math (per batch b):
    h = GroupNorm32(x) * gamma + beta               # [C, N], C=256, N=1024
    q = wq @ h + bq ; k = wk @ h + bk ; v = wv @ h + bv
    s[m, n] = <q[:, m], k[:, n]> / sqrt(C)
    w = softmax(s, axis=n)
    o[c, m] = sum_n w[m, n] v[c, n]
    out = x + wp @ o + bp

Device-side layout strategy (per core: 4 batches):
  - q, k in natural [c, n] layout; scores computed TRANSPOSED
    (sT[n, m] = k^T q) so exp(sT) is already partition-major in n — the
    contraction axis of the attend matmul — avoiding any 128x128 transposes.
  - Softmax runs without max-subtraction (scores are ~N(0,1); exp is safe in
    fp32) so exp comes straight off the scores PSUM.
  - Row sums via a ones[128,128] stationary matmul, which also broadcasts the
    denominators to all partitions for free. 1/x via the custom-DVE
    reciprocal_approx_accurate; GroupNorm rstd via a DVE Newton rsqrt. ScalarE
    then only ever runs Exp/Identity (one table set, no table-switch stalls).
  - v is computed directly transposed (vT = h^T @ wvT) and the attend output
    lands back in natural [c, m] layout; softmax normalization is folded into
    the attend PSUM eviction; proj bias + residual are folded into the final
    eviction (scalar_tensor_tensor).
  - All big matmuls run as float32r (full PE rate at free-dim >= 256), fp32
    storage and PSUM accumulation everywhere.
"""

import sys

sys.path.insert(0, "/opt/trn_rl_repo")

import numpy as np

import concourse.bass as bass
import concourse.tile as tile
from concourse import bacc, mybir

F32 = mybir.dt.float32
F32R = mybir.dt.float32r
AF = mybir.ActivationFunctionType
OP = mybir.AluOpType

N_CORES = 8
B = 32  # full batch
B_LOC = B // N_CORES  # batches per core
C = 256
CT = 2  # channel tiles of 128
N = 1024  # spatial (32*32)
NT = 8  # spatial partition-tiles of 128
MCH = 2  # spatial free-dim chunks of 512
GPC = 16  # groups per channel tile (32 groups total)
EPS = 1e-5
SCALE = C ** -0.5  # 1/16


def _bcast_ap(handle, nparts):
    """Partition-broadcast read AP for a 1-D DRAM tensor."""
    ap = handle[:]
    return bass.AP(tensor=ap.tensor, offset=ap.offset, ap=[[0, nparts]] + list(ap.ap))


def _build_nc():
    nc = bacc.Bacc()

    x_d = nc.declare_dram_parameter("x", [B_LOC, C, N], F32, isOutput=False)
    wq_d = nc.declare_dram_parameter("wqT", [C, C], F32, isOutput=False)
    wk_d = nc.declare_dram_parameter("wkT", [C, C], F32, isOutput=False)
    wv_d = nc.declare_dram_parameter("wvT", [C, C], F32, isOutput=False)
    wp_d = nc.declare_dram_parameter("wpT", [C, C], F32, isOutput=False)
    gam_d = nc.declare_dram_parameter("gamma", [C], F32, isOutput=False)
    bet_d = nc.declare_dram_parameter("beta", [C], F32, isOutput=False)
    bq_d = nc.declare_dram_parameter("bq", [C], F32, isOutput=False)
    bk_d = nc.declare_dram_parameter("bk", [C], F32, isOutput=False)
    bv_d = nc.declare_dram_parameter("bv", [C], F32, isOutput=False)
    bp_d = nc.declare_dram_parameter("bp", [C], F32, isOutput=False)
    ones_d = nc.declare_dram_parameter("ones", [128], F32, isOutput=False)
    g8_d = nc.declare_dram_parameter("g8", [C, 2 * GPC], F32, isOutput=False)
    gt_d = nc.declare_dram_parameter("gt", [2 * GPC, C], F32, isOutput=False)
    out_d = nc.declare_dram_parameter("out", [B_LOC, C, N], F32, isOutput=True)

    with tile.TileContext(nc) as tc:
        with (
            tc.tile_pool(name="consts", bufs=1) as consts,
            tc.tile_pool(name="big", bufs=2) as big,
            tc.tile_pool(name="vtp", bufs=2) as vtp,
            tc.tile_pool(name="ptp", bufs=1) as ptp,
            tc.tile_pool(name="misc", bufs=2) as misc,
            tc.tile_pool(name="small", bufs=3) as small,
            tc.tile_pool(name="ps_a", bufs=2, space="PSUM") as ps_a,
            tc.tile_pool(name="ps_rs", bufs=1, space="PSUM") as ps_rs,
            tc.tile_pool(name="ps_m", bufs=2, space="PSUM") as ps_m,
        ):
            # ---------------- constants / weights (once) ----------------
            w_tiles = {}
            for nm, d in (("wq", wq_d), ("wk", wk_d), ("wv", wv_d), ("wp", wp_d)):
                t = consts.tile([128, CT, C], F32R, name=f"{nm}_t")
                for ci in range(CT):
                    nc.sync.dma_start(
                        out=t[:, ci, :],
                        in_=d[ci * 128 : (ci + 1) * 128, :].bitcast(F32R),
                    )
                w_tiles[nm] = t
            wq_t, wk_t, wv_t, wp_t = (
                w_tiles["wq"], w_tiles["wk"], w_tiles["wv"], w_tiles["wp"],
            )

            g8_t = consts.tile([128, CT, 2 * GPC], F32R, name="g8_t")
            for ci in range(CT):
                nc.sync.dma_start(
                    out=g8_t[:, ci, :],
                    in_=g8_d[ci * 128 : (ci + 1) * 128, :].bitcast(F32R),
                )
            gt_t = consts.tile([2 * GPC, CT, 128], F32R, name="gt_t")
            for ct in range(CT):
                nc.sync.dma_start(
                    out=gt_t[:, ct, :],
                    in_=gt_d[:, ct * 128 : (ct + 1) * 128].bitcast(F32R),
                )

            ones_t = consts.tile([128, 128], F32R, name="ones_t")
            nc.sync.dma_start(
                out=ones_t[:],
                in_=bass.AP(
                    tensor=ones_d[:].tensor,
                    offset=ones_d[:].offset,
                    ap=[[0, 128]] + list(ones_d[:].ap),
                ).bitcast(F32R),
            )

            bvb_t = consts.tile([128, C], F32, name="bvb_t")
            nc.sync.dma_start(out=bvb_t[:], in_=_bcast_ap(bv_d, 128))

            vec_tiles = {}
            for nm, d in (
                ("gam", gam_d), ("bet", bet_d), ("bq", bq_d), ("bk", bk_d), ("bp", bp_d),
            ):
                t = consts.tile([128, CT], F32, name=f"{nm}_v")
                for ci in range(CT):
                    nc.sync.dma_start(
                        out=t[:, ci : ci + 1],
                        in_=d[ci * 128 : (ci + 1) * 128][:, None],
                    )
                vec_tiles[nm] = t
            gam_t, bet_t = vec_tiles["gam"], vec_tiles["bet"]
            bq_t, bk_t, bp_t = vec_tiles["bq"], vec_tiles["bk"], vec_tiles["bp"]

            # ---------------- per-batch stages ----------------
            def load(b):
                s = {}
                xt = big.tile([128, CT, N], F32, name="xT")
                for ct in range(CT):
                    nc.sync.dma_start(
                        out=xt[:, ct, :], in_=x_d[b, ct * 128 : (ct + 1) * 128, :]
                    )
                s["x"] = xt
                return s

            def stage_a(s):
                """GroupNorm -> h; q,k projections; vT projection."""
                xt = s["x"]
                ht = big.tile([128, CT, N], F32R, name="hT")
                # per-channel [mean, E[x^2]] for the group-combine matmul
                st2s = []
                for ct in range(CT):
                    xin = xt[:, ct, :].rearrange("p (s f) -> p s f", f=512)
                    st6 = small.tile([128, 2, 6], F32, name="st6")
                    for sg in range(2):
                        nc.vector.bn_stats(out=st6[:, sg, :], in_=xin[:, sg, :])
                    mv = small.tile([128, 2], F32, name="mv")
                    nc.vector.bn_aggr(out=mv[:], in_=st6[:])
                    st2 = small.tile([128, 2], F32R, name=f"st2_{ct}")
                    nc.vector.tensor_copy(out=st2[:, 0:1], in_=mv[:, 0:1])
                    sq = small.tile([128, 1], F32, name="sq")
                    nc.vector.tensor_mul(out=sq[:], in0=mv[:, 0:1], in1=mv[:, 0:1])
                    nc.vector.tensor_add(out=st2[:, 1:2], in0=mv[:, 1:2], in1=sq[:])
                    st2s.append(st2)
                # all 32 groups at once: [32, 2] = sum_ci g8[ci]^T st2[ci]
                gsp = ps_m.tile([2 * GPC, 2], F32, name="gsp", tag="mm512")
                for ci in range(CT):
                    nc.tensor.matmul(
                        gsp[:], g8_t[:, ci, :], st2s[ci][:],
                        start=(ci == 0), stop=(ci == CT - 1),
                    )
                gss = small.tile([2 * GPC, 2], F32, name="gss")
                nc.vector.tensor_copy(out=gss[:], in_=gsp[:])
                # group var = E[x^2] - mean^2 + eps
                gsq = small.tile([2 * GPC, 1], F32, name="gsq")
                nc.vector.tensor_mul(out=gsq[:], in0=gss[:, 0:1], in1=gss[:, 0:1])
                gv = small.tile([2 * GPC, 1], F32, name="gv")
                nc.vector.scalar_tensor_tensor(
                    out=gv[:], in0=gsq[:], scalar=-1.0, in1=gss[:, 1:2],
                    op0=OP.mult, op1=OP.add,
                )
                gve = small.tile([2 * GPC, 1], F32, name="gve")
                nc.vector.tensor_scalar_add(gve[:], gv[:], EPS)
                # rstd = rsqrt(gve): Newton from seed min(1/v, 1)
                rc = small.tile([2 * GPC, 1], F32, name="rc")
                nc.vector.reciprocal(out=rc[:], in_=gve[:])
                r = small.tile([2 * GPC, 1], F32, name="rn0")
                nc.vector.tensor_scalar_min(r[:], rc[:], 1.0)
                for it in range(4):
                    t1 = small.tile([2 * GPC, 1], F32, name="nw_t1")
                    nc.vector.tensor_mul(out=t1[:], in0=r[:], in1=r[:])
                    t2 = small.tile([2 * GPC, 1], F32, name="nw_t2")
                    nc.vector.scalar_tensor_tensor(
                        out=t2[:], in0=t1[:], scalar=-0.5, in1=gve[:],
                        op0=OP.mult, op1=OP.mult,
                    )
                    rn = small.tile([2 * GPC, 1], F32, name="nw_r")
                    nc.vector.scalar_tensor_tensor(
                        out=rn[:], in0=t2[:], scalar=1.5, in1=r[:],
                        op0=OP.add, op1=OP.mult,
                    )
                    r = rn
                sg2 = small.tile([2 * GPC, 2], F32R, name="sg2")
                nc.vector.tensor_copy(out=sg2[:, 0:1], in_=gss[:, 0:1])
                nc.vector.tensor_copy(out=sg2[:, 1:2], in_=r[:])
                for ct in range(CT):
                    # broadcast group (mean, rstd) back to channels
                    csp = ps_m.tile([128, 2], F32, name="csp", tag="mm512")
                    nc.tensor.matmul(
                        csp[:], gt_t[:, ct, :], sg2[:], start=True, stop=True
                    )
                    # A = rstd*gamma ; B = beta - mean*A ; h = x*A + B
                    a_t = small.tile([128, 1], F32, name="a_vec")
                    b_t = small.tile([128, 1], F32, name="b_vec")
                    nc.vector.tensor_mul(
                        out=a_t[:], in0=csp[:, 1:2], in1=gam_t[:, ct : ct + 1]
                    )
                    tmb = small.tile([128, 1], F32, name="tmb")
                    nc.vector.tensor_mul(out=tmb[:], in0=csp[:, 0:1], in1=a_t[:])
                    nc.vector.tensor_sub(
                        out=b_t[:], in0=bet_t[:, ct : ct + 1], in1=tmb[:]
                    )
                    nc.vector.tensor_scalar(
                        ht[:, ct, :], xt[:, ct, :], a_t[:], b_t[:], OP.mult, OP.add
                    )
                s["h"] = ht

                # q, k: [co, m] = sum_ci wT[ci, co]^T h[ci, m]  (+ bias on evict)
                qt = big.tile([128, CT, N], F32R, name="qT")
                kt = big.tile([128, CT, N], F32R, name="kT")
                for dst, w_t, bias_t in ((qt, wq_t, bq_t), (kt, wk_t, bk_t)):
                    for co in range(CT):
                        acc = ps_a.tile([128, N], F32, name="acc", tag="acc")
                        for mch in range(MCH):
                            msl = slice(mch * 512, (mch + 1) * 512)
                            for ci in range(CT):
                                nc.tensor.matmul(
                                    acc[:, msl],
                                    w_t[:, ci, co * 128 : (co + 1) * 128],
                                    dst_rhs := s["h"][:, ci, msl],
                                    start=(ci == 0),
                                    stop=(ci == CT - 1),
                                )
                        nc.scalar.activation(
                            out=dst[:, co, :], in_=acc[:], func=AF.Identity,
                            bias=bias_t[:, co : co + 1], scale=1.0,
                        )
                s["q"], s["k"] = qt, kt

                # vT: [n_tile, c] = h[ci, n_tile]^T wvT[ci, :]  (+ bv on evict)
                vts = []
                for nt in range(NT):
                    vp = ps_m.tile([128, C], F32, name="vp", tag="mm512")
                    for ci in range(CT):
                        nc.tensor.matmul(
                            vp[:],
                            s["h"][:, ci, nt * 128 : (nt + 1) * 128],
                            wv_t[:, ci, :],
                            start=(ci == 0),
                            stop=(ci == CT - 1),
                        )
                    vt = vtp.tile([128, C], F32R, name=f"vt{nt}")
                    nc.vector.tensor_add(out=vt[:], in0=vp[:], in1=bvb_t[:])
                    vts.append(vt)
                s["v"] = vts

            def stage_b(s):
                """scores^T -> exp -> pT ; row sums (interleaved per n-tile)."""
                rs = ps_rs.tile([128, N], F32, name="rsp")
                pts = []
                for nt in range(NT):
                    stp = ps_a.tile([128, N], F32, name="stp", tag="acc")
                    for mch in range(MCH):
                        msl = slice(mch * 512, (mch + 1) * 512)
                        for ci in range(CT):
                            nc.tensor.matmul(
                                stp[:, msl],
                                s["k"][:, ci, nt * 128 : (nt + 1) * 128],
                                s["q"][:, ci, msl],
                                start=(ci == 0),
                                stop=(ci == CT - 1),
                            )
                    pt = ptp.tile([128, N], F32R, name=f"pt{nt}")
                    nc.scalar.activation(
                        out=pt[:], in_=stp[:], func=AF.Exp, bias=0.0, scale=SCALE
                    )
                    pts.append(pt)
                    for mch in range(MCH):
                        msl = slice(mch * 512, (mch + 1) * 512)
                        nc.tensor.matmul(
                            rs[:, msl], ones_t[:], pt[:, msl],
                            start=(nt == 0), stop=(nt == NT - 1),
                        )
                s["p"] = pts
                s["rs"] = rs

            def stage_c(s, b):
                """1/rowsum; attend (+normalize); project (+bias+residual); store."""
                scr = misc.tile([128, N], F32, name="scr")
                rcp = misc.tile([128, N], F32, name="rcp")
                nc.vector.reciprocal_approx_accurate(
                    out=rcp[:], in_=s["rs"][:], scratch=scr[:]
                )

                ont = big.tile([128, CT, N], F32R, name="onT")
                for ct in range(CT):
                    for mch in range(MCH):
                        msl = slice(mch * 512, (mch + 1) * 512)
                        ap_ = ps_m.tile([128, 512], F32, name="attp", tag="mm512")
                        for nt in range(NT):
                            nc.tensor.matmul(
                                ap_[:],
                                s["v"][nt][:, ct * 128 : (ct + 1) * 128],
                                s["p"][nt][:, msl],
                                start=(nt == 0),
                                stop=(nt == NT - 1),
                            )
                        nc.vector.tensor_mul(
                            out=ont[:, ct, msl], in0=ap_[:], in1=rcp[:, msl]
                        )

                outf = big.tile([128, CT, N], F32, name="outf")
                for co in range(CT):
                    for mch in range(MCH):
                        msl = slice(mch * 512, (mch + 1) * 512)
                        pp = ps_m.tile([128, 512], F32, name="pp", tag="mm512")
                        for ci in range(CT):
                            nc.tensor.matmul(
                                pp[:],
                                wp_t[:, ci, co * 128 : (co + 1) * 128],
                                ont[:, ci, msl],
                                start=(ci == 0),
                                stop=(ci == CT - 1),
                            )
                        nc.vector.scalar_tensor_tensor(
                            out=outf[:, co, msl],
                            in0=pp[:],
                            scalar=bp_t[:, co : co + 1],
                            in1=s["x"][:, co, msl],
                            op0=OP.add,
                            op1=OP.add,
                        )
                    nc.sync.dma_start(
                        out=out_d[b, co * 128 : (co + 1) * 128, :], in_=outf[:, co, :]
                    )

            # Interleaved emission: A(b+1) slots between B(b) and C(b) so PE
            # has dense work while ACT runs batch b's exp pass.
            states = [None] * B_LOC
            states[0] = load(0)
            stage_a(states[0])
            stage_b(states[0])
            for b in range(B_LOC):
                if b + 1 < B_LOC:
                    states[b + 1] = load(b + 1)
                    stage_a(states[b + 1])
                stage_c(states[b], b)
                states[b] = None
                if b + 1 < B_LOC:
                    stage_b(states[b + 1])

    nc.finalize()
    return nc


_NC = None


def _get_nc():
    global _NC
    if _NC is None:
        _NC = _build_nc()
    return _NC


def _make_in_maps(inputs):
    x = np.asarray(inputs["x"], dtype=np.float32).reshape(B, C, N)
    g8 = np.zeros((C, 2 * GPC), np.float32)
    for c in range(C):
        g8[c, c // 8] = 0.125
    gt = np.zeros((2 * GPC, C), np.float32)
    for c in range(C):
        gt[c // 8, c] = 1.0

    shared = {
        "wqT": np.ascontiguousarray(np.asarray(inputs["wq"], np.float32).T),
        "wkT": np.ascontiguousarray(np.asarray(inputs["wk"], np.float32).T),
        "wvT": np.ascontiguousarray(np.asarray(inputs["wv"], np.float32).T),
        "wpT": np.ascontiguousarray(np.asarray(inputs["wp"], np.float32).T),
        "gamma": np.asarray(inputs["gamma"], np.float32),
        "beta": np.asarray(inputs["beta"], np.float32),
        "bq": np.asarray(inputs["bq"], np.float32),
        "bk": np.asarray(inputs["bk"], np.float32),
        "bv": np.asarray(inputs["bv"], np.float32),
        "bp": np.asarray(inputs["bp"], np.float32),
        "g8": g8,
        "gt": gt,
        "ones": np.ones((128,), np.float32),
    }
    in_maps = []
    for i in range(N_CORES):
        m = dict(shared)
        m["x"] = np.ascontiguousarray(x[i * B_LOC : (i + 1) * B_LOC])
        in_maps.append(m)
    return in_maps


def _run(inputs, trace=False):
    from concourse.bass_utils import run_bass_kernel_spmd

    nc = _get_nc()
    in_maps = _make_in_maps(inputs)
    res = run_bass_kernel_spmd(
        nc, in_maps, core_ids=list(range(N_CORES)), trace=trace
    )
    out = np.concatenate([r["out"] for r in res.results], axis=0)
    return out.reshape(B, C, 32, 32).astype(np.float32), res


def kernel(**inputs) -> np.ndarray:
    out, _ = _run(inputs, trace=False)
    return out


# revision 15
# speedup vs baseline: 1.1695x; 1.1695x over previous
"""AttnBlock (GroupNorm + single-head self-attention + residual) on 8 Trainium2
NeuronCores, pure data-parallel over the batch dimension.

Reference math (per batch b):
    h = GroupNorm32(x) * gamma + beta               # [C, N], C=256, N=1024
    q = wq @ h + bq ; k = wk @ h + bk ; v = wv @ h + bv
    s[m, n] = <q[:, m], k[:, n]> / sqrt(C)
    w = softmax(s, axis=n)
    o[c, m] = sum_n w[m, n] v[c, n]
    out = x + wp @ o + bp

Device-side strategy (per core: 4 batches):
  - q, k in natural [c, n] layout; scores computed TRANSPOSED
    (sT[n, m] = k^T q) so exp(sT) is already partition-major in n — the
    contraction axis of the attend matmul — avoiding any 128x128 transposes.
  - Softmax runs without max-subtraction (scores are ~N(0,1); exp is safe in
    fp32) so exp comes straight off the scores PSUM.
  - Row sums via a ones[128,128] stationary matmul, which also broadcasts the
    denominators to all partitions for free. 1/x via the custom-DVE
    reciprocal_approx_accurate; GroupNorm rstd via a DVE Newton rsqrt. ScalarE
    then only ever runs Exp/Identity (one table set, no table-switch stalls).
  - v is computed directly transposed (vT = h^T @ wvT); softmax normalization
    is folded into the attend PSUM eviction; proj bias + residual are folded
    into the final eviction (scalar_tensor_tensor).
  - All big matmuls run as float32r (full PE rate at free-dim >= 256), fp32
    storage and PSUM accumulation everywhere.
  - Emission interleaves batches: batch b+1's GroupNorm stat chain (serial
    small DVE ops) hides under batch b's scores/exp phase; batch b+1's
    h/q/k/vT projections fill the PE while batch b's exp tail finishes.
"""

import sys

sys.path.insert(0, "/opt/trn_rl_repo")

import numpy as np

import concourse.bass as bass
import concourse.tile as tile
from concourse import bacc, mybir

F32 = mybir.dt.float32
F32R = mybir.dt.float32r
AF = mybir.ActivationFunctionType
OP = mybir.AluOpType

N_CORES = 8
B = 32  # full batch
B_LOC = B // N_CORES  # batches per core
C = 256
CT = 2  # channel tiles of 128
N = 1024  # spatial (32*32)
NT = 8  # spatial partition-tiles of 128
MCH = 2  # spatial free-dim chunks of 512
G = 32  # groups
EPS = 1e-5
SCALE = C ** -0.5  # 1/16


def _bcast_ap(handle, nparts):
    """Partition-broadcast read AP for a 1-D DRAM tensor."""
    ap = handle[:]
    return bass.AP(tensor=ap.tensor, offset=ap.offset, ap=[[0, nparts]] + list(ap.ap))


def _build_nc():
    nc = bacc.Bacc()

    x_d = nc.declare_dram_parameter("x", [B_LOC, C, N], F32, isOutput=False)
    wq_d = nc.declare_dram_parameter("wqT", [C, C], F32, isOutput=False)
    wk_d = nc.declare_dram_parameter("wkT", [C, C], F32, isOutput=False)
    wv_d = nc.declare_dram_parameter("wvT", [C, C], F32, isOutput=False)
    wp_d = nc.declare_dram_parameter("wpT", [C, C], F32, isOutput=False)
    vec_d = nc.declare_dram_parameter("vecs", [5, C], F32, isOutput=False)
    bv_d = nc.declare_dram_parameter("bv", [C], F32, isOutput=False)
    ones_d = nc.declare_dram_parameter("ones", [128], F32, isOutput=False)
    g8_d = nc.declare_dram_parameter("g8", [C, G], F32, isOutput=False)
    gt_d = nc.declare_dram_parameter("gt", [G, C], F32, isOutput=False)
    out_d = nc.declare_dram_parameter("out", [B_LOC, C, N], F32, isOutput=True)

    with tile.TileContext(nc) as tc:
        with (
            tc.tile_pool(name="consts", bufs=1) as consts,
            tc.tile_pool(name="big", bufs=2) as big,
            tc.tile_pool(name="vtp", bufs=2) as vtp,
            tc.tile_pool(name="ptp", bufs=1) as ptp,
            tc.tile_pool(name="misc", bufs=2) as misc,
            tc.tile_pool(name="small", bufs=3) as small,
            tc.tile_pool(name="ps_a", bufs=2, space="PSUM") as ps_a,
            tc.tile_pool(name="ps_rs", bufs=1, space="PSUM") as ps_rs,
            tc.tile_pool(name="ps_m", bufs=2, space="PSUM") as ps_m,
        ):
            # ------- batch-0 input load first: nothing queues ahead of it
            def load(b):
                s = {"b": b}
                xt = big.tile([128, CT, N], F32, name="xT")
                nc.sync.dma_start(
                    out=xt[:], in_=x_d[b].rearrange("(ct p) n -> p ct n", p=128)
                )
                s["x"] = xt
                return s

            states = [None] * B_LOC
            states[0] = load(0)

            # ------- constants / weights (one DMA each, gpsimd queue)
            w_tiles = {}
            for nm, d in (("wq", wq_d), ("wk", wk_d), ("wv", wv_d), ("wp", wp_d)):
                t = consts.tile([128, CT, C], F32R, name=f"{nm}_t")
                nc.gpsimd.dma_start(
                    out=t[:],
                    in_=d[:, :].rearrange("(ci p) o -> p ci o", p=128).bitcast(F32R),
                )
                w_tiles[nm] = t
            wq_t, wk_t, wv_t, wp_t = (
                w_tiles["wq"], w_tiles["wk"], w_tiles["wv"], w_tiles["wp"],
            )

            # gamma, beta, bq, bk, bp as per-partition [128, 5, CT]
            vec_t = consts.tile([128, 5, CT], F32, name="vec_t")
            nc.gpsimd.dma_start(
                out=vec_t[:], in_=vec_d[:, :].rearrange("v (ct p) -> p v ct", p=128)
            )
            GAM, BET, BQ, BK, BP = range(5)

            g8_t = consts.tile([128, CT, G], F32R, name="g8_t")
            nc.gpsimd.dma_start(
                out=g8_t[:],
                in_=g8_d[:, :].rearrange("(ci p) g -> p ci g", p=128).bitcast(F32R),
            )
            gt_t = consts.tile([G, CT, 128], F32R, name="gt_t")
            nc.gpsimd.dma_start(
                out=gt_t[:],
                in_=gt_d[:, :].rearrange("g (ct p) -> g ct p", p=128).bitcast(F32R),
            )
            ones_t = consts.tile([128, 128], F32R, name="ones_t")
            nc.gpsimd.dma_start(
                out=ones_t[:], in_=_bcast_ap(ones_d, 128).bitcast(F32R)
            )
            bvb_t = consts.tile([128, C], F32, name="bvb_t")
            nc.gpsimd.dma_start(out=bvb_t[:], in_=_bcast_ap(bv_d, 128))

            # ---------------- per-batch stages ----------------

            def gn_pre(s):
                """bn stats -> per-channel [mean, E[x^2]+eps] -> group stats
                -> Newton rsqrt -> sg2 = [mean_g, rstd_g]. Mostly small serial
                DVE work; emitted where PE is busy with the previous batch."""
                xt = s["x"]
                st2s = []
                for ct in range(CT):
                    xin = xt[:, ct, :].rearrange("p (s f) -> p s f", f=512)
                    st6 = small.tile([128, 2, 6], F32, name="st6")
                    for sg in range(2):
                        nc.vector.bn_stats(out=st6[:, sg, :], in_=xin[:, sg, :])
                    mv = small.tile([128, 2], F32, name="mv")
                    nc.vector.bn_aggr(out=mv[:], in_=st6[:])
                    st2 = small.tile([128, 2], F32R, name=f"st2_{ct}")
                    nc.vector.tensor_copy(out=st2[:, 0:1], in_=mv[:, 0:1])
                    sq = small.tile([128, 1], F32, name="sq")
                    nc.vector.tensor_mul(out=sq[:], in0=mv[:, 0:1], in1=mv[:, 0:1])
                    # col1 = E[x^2] + eps  (G8 rows sum to 1, so eps survives)
                    nc.vector.scalar_tensor_tensor(
                        out=st2[:, 1:2], in0=sq[:], scalar=EPS, in1=mv[:, 1:2],
                        op0=OP.add, op1=OP.add,
                    )
                    st2s.append(st2)
                gsp = ps_m.tile([G, 2], F32, name="gsp", tag="mm512")
                for ci in range(CT):
                    nc.tensor.matmul(
                        gsp[:], g8_t[:, ci, :], st2s[ci][:],
                        start=(ci == 0), stop=(ci == CT - 1),
                    )
                gss = small.tile([G, 2], F32, name="gss")
                nc.vector.tensor_copy(out=gss[:], in_=gsp[:])
                # v = (E[x^2]+eps) - mean^2 ; rstd = rsqrt(v)
                gsq = small.tile([G, 1], F32, name="gsq")
                nc.vector.tensor_mul(out=gsq[:], in0=gss[:, 0:1], in1=gss[:, 0:1])
                gv = small.tile([G, 1], F32, name="gv")
                nc.vector.scalar_tensor_tensor(
                    out=gv[:], in0=gsq[:], scalar=-1.0, in1=gss[:, 1:2],
                    op0=OP.mult, op1=OP.add,
                )
                rc = small.tile([G, 1], F32, name="rc")
                nc.vector.reciprocal(out=rc[:], in_=gv[:])
                r = small.tile([G, 1], F32, name="rn0")
                nc.vector.tensor_scalar_min(r[:], rc[:], 1.0)
                sg2 = small.tile([G, 2], F32R, name="sg2")
                nc.vector.tensor_copy(out=sg2[:, 0:1], in_=gss[:, 0:1])
                for it in range(3):
                    t1 = small.tile([G, 1], F32, name="nw_t1")
                    nc.vector.tensor_mul(out=t1[:], in0=r[:], in1=r[:])
                    t2 = small.tile([G, 1], F32, name="nw_t2")
                    nc.vector.scalar_tensor_tensor(
                        out=t2[:], in0=t1[:], scalar=-0.5, in1=gv[:],
                        op0=OP.mult, op1=OP.mult,
                    )
                    dst = sg2[:, 1:2] if it == 2 else small.tile(
                        [G, 1], F32, name="nw_r"
                    )
                    nc.vector.scalar_tensor_tensor(
                        out=dst, in0=t2[:], scalar=1.5, in1=r[:],
                        op0=OP.add, op1=OP.mult,
                    )
                    if it < 2:
                        r = dst
                s["sg2"] = sg2

            def gn_post(s):
                """Broadcast group stats to channels; per-channel affine
                A = rstd*gamma, B2 = mean*A - beta (h computed as x*A - B2)."""
                a_t = small.tile([128, CT], F32, name="a_vec")
                b2_t = small.tile([128, CT], F32, name="b2_vec")
                for ct in range(CT):
                    csp = ps_m.tile([128, 2], F32, name="csp", tag="mm512")
                    nc.tensor.matmul(
                        csp[:], gt_t[:, ct, :], s["sg2"][:], start=True, stop=True
                    )
                    nc.vector.tensor_mul(
                        out=a_t[:, ct : ct + 1], in0=csp[:, 1:2],
                        in1=vec_t[:, GAM, ct : ct + 1],
                    )
                    nc.vector.scalar_tensor_tensor(
                        out=b2_t[:, ct : ct + 1], in0=csp[:, 0:1],
                        scalar=a_t[:, ct : ct + 1], in1=vec_t[:, BET, ct : ct + 1],
                        op0=OP.mult, op1=OP.subtract,
                    )
                s["a"], s["b2"] = a_t, b2_t

            def stage_proj(s):
                """h = x*A - B2 ; q,k (natural) ; vT (transposed) projections."""
                ht = big.tile([128, CT, N], F32R, name="hT")
                for ct in range(CT):
                    nc.vector.tensor_scalar(
                        ht[:, ct, :], s["x"][:, ct, :],
                        s["a"][:, ct : ct + 1], s["b2"][:, ct : ct + 1],
                        OP.mult, OP.subtract,
                    )
                s["h"] = ht

                qt = big.tile([128, CT, N], F32R, name="qT")
                kt = big.tile([128, CT, N], F32R, name="kT")
                for dst, w_t, bias_idx in ((qt, wq_t, BQ), (kt, wk_t, BK)):
                    for co in range(CT):
                        acc = ps_a.tile([128, N], F32, name="acc", tag="acc")
                        for mch in range(MCH):
                            msl = slice(mch * 512, (mch + 1) * 512)
                            for ci in range(CT):
                                nc.tensor.matmul(
                                    acc[:, msl],
                                    w_t[:, ci, co * 128 : (co + 1) * 128],
                                    ht[:, ci, msl],
                                    start=(ci == 0),
                                    stop=(ci == CT - 1),
                                )
                        nc.scalar.activation(
                            out=dst[:, co, :], in_=acc[:], func=AF.Identity,
                            bias=vec_t[:, bias_idx, co : co + 1], scale=1.0,
                        )
                s["q"], s["k"] = qt, kt

                vts = []
                for nt in range(NT):
                    vp = ps_m.tile([128, C], F32, name="vp", tag="mm512")
                    for ci in range(CT):
                        nc.tensor.matmul(
                            vp[:],
                            ht[:, ci, nt * 128 : (nt + 1) * 128],
                            wv_t[:, ci, :],
                            start=(ci == 0),
                            stop=(ci == CT - 1),
                        )
                    vt = vtp.tile([128, C], F32R, name=f"vt{nt}")
                    nc.vector.tensor_add(out=vt[:], in0=vp[:], in1=bvb_t[:])
                    vts.append(vt)
                s["v"] = vts

            def stage_b(s, nxt):
                """scores^T -> exp -> pT ; row sums; next batch's gn chain
                interleaved so its serial DVE latency hides under PE work."""
                rs = ps_rs.tile([128, N], F32, name="rsp")
                pts = []
                for nt in range(NT):
                    stp = ps_a.tile([128, N], F32, name="stp", tag="acc")
                    for mch in range(MCH):
                        msl = slice(mch * 512, (mch + 1) * 512)
                        for ci in range(CT):
                            nc.tensor.matmul(
                                stp[:, msl],
                                s["k"][:, ci, nt * 128 : (nt + 1) * 128],
                                s["q"][:, ci, msl],
                                start=(ci == 0),
                                stop=(ci == CT - 1),
                            )
                    pt = ptp.tile([128, N], F32R, name=f"pt{nt}")
                    nc.scalar.activation(
                        out=pt[:], in_=stp[:], func=AF.Exp, bias=0.0, scale=SCALE
                    )
                    pts.append(pt)
                    for mch in range(MCH):
                        msl = slice(mch * 512, (mch + 1) * 512)
                        nc.tensor.matmul(
                            rs[:, msl], ones_t[:], pt[:, msl],
                            start=(nt == 0), stop=(nt == NT - 1),
                        )
                    if nt == 5 and nxt is not None:
                        gn_pre(nxt)
                s["p"] = pts
                s["rs"] = rs
                if nxt is not None:
                    gn_post(nxt)

            def stage_c(s):
                """1/rowsum; attend (+normalize); project (+bias+residual)."""
                scr = misc.tile([128, N], F32, name="scr")
                rcp = misc.tile([128, N], F32, name="rcp")
                nc.vector.reciprocal_approx_accurate(
                    out=rcp[:], in_=s["rs"][:], scratch=scr[:]
                )

                ont = big.tile([128, CT, N], F32R, name="onT")
                for ct in range(CT):
                    for mch in range(MCH):
                        msl = slice(mch * 512, (mch + 1) * 512)
                        ap_ = ps_m.tile([128, 512], F32, name="attp", tag="mm512")
                        for nt in range(NT):
                            nc.tensor.matmul(
                                ap_[:],
                                s["v"][nt][:, ct * 128 : (ct + 1) * 128],
                                s["p"][nt][:, msl],
                                start=(nt == 0),
                                stop=(nt == NT - 1),
                            )
                        nc.vector.tensor_mul(
                            out=ont[:, ct, msl], in0=ap_[:], in1=rcp[:, msl]
                        )

                outf = big.tile([128, CT, N], F32, name="outf")
                for co in range(CT):
                    for mch in range(MCH):
                        msl = slice(mch * 512, (mch + 1) * 512)
                        pp = ps_m.tile([128, 512], F32, name="pp", tag="mm512")
                        for ci in range(CT):
                            nc.tensor.matmul(
                                pp[:],
                                wp_t[:, ci, co * 128 : (co + 1) * 128],
                                ont[:, ci, msl],
                                start=(ci == 0),
                                stop=(ci == CT - 1),
                            )
                        nc.vector.scalar_tensor_tensor(
                            out=outf[:, co, msl],
                            in0=pp[:],
                            scalar=vec_t[:, BP, co : co + 1],
                            in1=s["x"][:, co, msl],
                            op0=OP.add,
                            op1=OP.add,
                        )
                nc.sync.dma_start(
                    out=out_d[s["b"]].rearrange("(ct p) n -> p ct n", p=128),
                    in_=outf[:],
                )

            # ---------------- emission schedule ----------------
            gn_pre(states[0])
            gn_post(states[0])
            stage_proj(states[0])
            for b in range(B_LOC):
                nxt = None
                if b + 1 < B_LOC:
                    states[b + 1] = load(b + 1)
                    nxt = states[b + 1]
                stage_b(states[b], nxt)
                stage_c(states[b])
                states[b] = None
                if nxt is not None:
                    stage_proj(nxt)

    nc.finalize()
    return nc


_NC = None


def _get_nc():
    global _NC
    if _NC is None:
        _NC = _build_nc()
    return _NC


def _make_in_maps(inputs):
    x = np.asarray(inputs["x"], dtype=np.float32).reshape(B, C, N)
    g8 = np.zeros((C, G), np.float32)
    for c in range(C):
        g8[c, c // 8] = 0.125
    gt = np.zeros((G, C), np.float32)
    for c in range(C):
        gt[c // 8, c] = 1.0
    vecs = np.stack(
        [
            np.asarray(inputs["gamma"], np.float32),
            np.asarray(inputs["beta"], np.float32),
            np.asarray(inputs["bq"], np.float32),
            np.asarray(inputs["bk"], np.float32),
            np.asarray(inputs["bp"], np.float32),
        ]
    )

    shared = {
        "wqT": np.ascontiguousarray(np.asarray(inputs["wq"], np.float32).T),
        "wkT": np.ascontiguousarray(np.asarray(inputs["wk"], np.float32).T),
        "wvT": np.ascontiguousarray(np.asarray(inputs["wv"], np.float32).T),
        "wpT": np.ascontiguousarray(np.asarray(inputs["wp"], np.float32).T),
        "vecs": vecs,
        "bv": np.asarray(inputs["bv"], np.float32),
        "g8": g8,
        "gt": gt,
        "ones": np.ones((128,), np.float32),
    }
    in_maps = []
    for i in range(N_CORES):
        m = dict(shared)
        m["x"] = np.ascontiguousarray(x[i * B_LOC : (i + 1) * B_LOC])
        in_maps.append(m)
    return in_maps


def _run(inputs, trace=False):
    from concourse.bass_utils import run_bass_kernel_spmd

    nc = _get_nc()
    in_maps = _make_in_maps(inputs)
    res = run_bass_kernel_spmd(
        nc, in_maps, core_ids=list(range(N_CORES)), trace=trace
    )
    out = np.concatenate([r["out"] for r in res.results], axis=0)
    return out.reshape(B, C, 32, 32).astype(np.float32), res


def kernel(**inputs) -> np.ndarray:
    out, _ = _run(inputs, trace=False)
    return out


# revision 16
# speedup vs baseline: 1.1933x; 1.0204x over previous
"""AttnBlock (GroupNorm + single-head self-attention + residual) on 8 Trainium2
NeuronCores, pure data-parallel over the batch dimension.

Reference math (per batch b):
    h = GroupNorm32(x) * gamma + beta               # [C, N], C=256, N=1024
    q = wq @ h + bq ; k = wk @ h + bk ; v = wv @ h + bv
    s[m, n] = <q[:, m], k[:, n]> / sqrt(C)
    w = softmax(s, axis=n)
    o[c, m] = sum_n w[m, n] v[c, n]
    out = x + wp @ o + bp

Device-side strategy (per core: 4 batches):
  - q, k in natural [c, n] layout; scores computed TRANSPOSED
    (sT[n, m] = k^T q) so exp(sT) is already partition-major in n — the
    contraction axis of the attend matmul — avoiding any 128x128 transposes.
  - Softmax runs without max-subtraction (scores are ~N(0,1); exp is safe in
    fp32) so exp comes straight off the scores PSUM.
  - Row sums via a ones[128,128] stationary matmul, which also broadcasts the
    denominators to all partitions for free. 1/x via the custom-DVE
    reciprocal_approx_accurate; GroupNorm rstd via a DVE Newton rsqrt. ScalarE
    then only ever runs Exp/Identity (one table set, no table-switch stalls).
  - v is computed directly transposed (vT = h^T @ wvT); softmax normalization
    is folded into the attend PSUM eviction; proj bias + residual are folded
    into the final eviction (scalar_tensor_tensor).
  - All big matmuls run as float32r (full PE rate at free-dim >= 256), fp32
    storage and PSUM accumulation everywhere.
  - Emission interleaves batches: batch b+1's GroupNorm stat chain (serial
    small DVE ops) hides under batch b's scores/exp phase; batch b+1's
    h/q/k/vT projections fill the PE while batch b's exp tail finishes.
"""

import sys

sys.path.insert(0, "/opt/trn_rl_repo")

import ml_dtypes
import numpy as np

import concourse.bass as bass
import concourse.tile as tile
from concourse import bacc, mybir

F32 = mybir.dt.float32
F32R = mybir.dt.float32r
BF16 = mybir.dt.bfloat16

# attention-path dtypes: p/v (attend + rowsum) in bf16 halves PE streaming
# time (XBUS carries 2 bf16/cycle); q/k bf16 additionally halves the scores
# matmuls. PSUM accumulation stays fp32 throughout.
PV_DT = BF16
QK_DT = BF16
AF = mybir.ActivationFunctionType
OP = mybir.AluOpType

N_CORES = 8
B = 32  # full batch
B_LOC = B // N_CORES  # batches per core
C = 256
CT = 2  # channel tiles of 128
N = 1024  # spatial (32*32)
NT = 8  # spatial partition-tiles of 128
MCH = 2  # spatial free-dim chunks of 512
G = 32  # groups
EPS = 1e-5
SCALE = C ** -0.5  # 1/16


def _bcast_ap(handle, nparts):
    """Partition-broadcast read AP for a 1-D DRAM tensor."""
    ap = handle[:]
    return bass.AP(tensor=ap.tensor, offset=ap.offset, ap=[[0, nparts]] + list(ap.ap))


def _build_nc():
    nc = bacc.Bacc()

    x_d = nc.declare_dram_parameter("x", [B_LOC, C, N], F32, isOutput=False)
    wq_d = nc.declare_dram_parameter("wqT", [C, C], F32, isOutput=False)
    wk_d = nc.declare_dram_parameter("wkT", [C, C], F32, isOutput=False)
    wv_d = nc.declare_dram_parameter("wvT", [C, C], F32, isOutput=False)
    wp_d = nc.declare_dram_parameter("wpT", [C, C], F32, isOutput=False)
    vec_d = nc.declare_dram_parameter("vecs", [5, C], F32, isOutput=False)
    bv_d = nc.declare_dram_parameter("bv", [C], F32, isOutput=False)
    ones_d = nc.declare_dram_parameter("ones", [128], PV_DT, isOutput=False)
    g8_d = nc.declare_dram_parameter("g8", [C, G], F32, isOutput=False)
    gt_d = nc.declare_dram_parameter("gt", [G, C], F32, isOutput=False)
    out_d = nc.declare_dram_parameter("out", [B_LOC, C, N], F32, isOutput=True)

    with tile.TileContext(nc) as tc:
        with (
            tc.tile_pool(name="consts", bufs=1) as consts,
            tc.tile_pool(name="big", bufs=2) as big,
            tc.tile_pool(name="vtp", bufs=2) as vtp,
            tc.tile_pool(name="ptp", bufs=2) as ptp,
            tc.tile_pool(name="misc", bufs=2) as misc,
            tc.tile_pool(name="small", bufs=3) as small,
            tc.tile_pool(name="ps_a", bufs=2, space="PSUM") as ps_a,
            tc.tile_pool(name="ps_rs", bufs=1, space="PSUM") as ps_rs,
            tc.tile_pool(name="ps_m", bufs=2, space="PSUM") as ps_m,
        ):
            # ------- batch-0 input load first: nothing queues ahead of it
            def load(b):
                s = {"b": b}
                xt = big.tile([128, CT, N], F32, name="xT")
                nc.sync.dma_start(
                    out=xt[:], in_=x_d[b].rearrange("(ct p) n -> p ct n", p=128)
                )
                s["x"] = xt
                return s

            states = [None] * B_LOC
            states[0] = load(0)

            # ------- constants / weights (one DMA each, gpsimd queue)
            w_tiles = {}
            for nm, d in (("wq", wq_d), ("wk", wk_d), ("wv", wv_d), ("wp", wp_d)):
                t = consts.tile([128, CT, C], F32R, name=f"{nm}_t")
                nc.gpsimd.dma_start(
                    out=t[:],
                    in_=d[:, :].rearrange("(ci p) o -> p ci o", p=128).bitcast(F32R),
                )
                w_tiles[nm] = t
            wq_t, wk_t, wv_t, wp_t = (
                w_tiles["wq"], w_tiles["wk"], w_tiles["wv"], w_tiles["wp"],
            )

            # gamma, beta, bq, bk, bp as per-partition [128, 5, CT]
            vec_t = consts.tile([128, 5, CT], F32, name="vec_t")
            nc.gpsimd.dma_start(
                out=vec_t[:], in_=vec_d[:, :].rearrange("v (ct p) -> p v ct", p=128)
            )
            GAM, BET, BQ, BK, BP = range(5)

            g8_t = consts.tile([128, CT, G], F32R, name="g8_t")
            nc.gpsimd.dma_start(
                out=g8_t[:],
                in_=g8_d[:, :].rearrange("(ci p) g -> p ci g", p=128).bitcast(F32R),
            )
            gt_t = consts.tile([G, CT, 128], F32R, name="gt_t")
            nc.gpsimd.dma_start(
                out=gt_t[:],
                in_=gt_d[:, :].rearrange("g (ct p) -> g ct p", p=128).bitcast(F32R),
            )
            ones_t = consts.tile([128, 128], PV_DT, name="ones_t")
            nc.gpsimd.dma_start(out=ones_t[:], in_=_bcast_ap(ones_d, 128))
            bvb_t = consts.tile([128, C], F32, name="bvb_t")
            nc.gpsimd.dma_start(out=bvb_t[:], in_=_bcast_ap(bv_d, 128))

            # ---------------- per-batch stages ----------------

            def gn_pre(s):
                """bn stats -> per-channel [mean, E[x^2]+eps] -> group stats
                -> Newton rsqrt -> sg2 = [mean_g, rstd_g]. Mostly small serial
                DVE work; emitted where PE is busy with the previous batch."""
                xt = s["x"]
                st2s = []
                for ct in range(CT):
                    xin = xt[:, ct, :].rearrange("p (s f) -> p s f", f=512)
                    st6 = small.tile([128, 2, 6], F32, name="st6")
                    for sg in range(2):
                        nc.vector.bn_stats(out=st6[:, sg, :], in_=xin[:, sg, :])
                    mv = small.tile([128, 2], F32, name="mv")
                    nc.vector.bn_aggr(out=mv[:], in_=st6[:])
                    st2 = small.tile([128, 2], F32R, name=f"st2_{ct}")
                    nc.vector.tensor_copy(out=st2[:, 0:1], in_=mv[:, 0:1])
                    sq = small.tile([128, 1], F32, name="sq")
                    nc.vector.tensor_mul(out=sq[:], in0=mv[:, 0:1], in1=mv[:, 0:1])
                    # col1 = E[x^2] + eps  (G8 rows sum to 1, so eps survives)
                    nc.vector.scalar_tensor_tensor(
                        out=st2[:, 1:2], in0=sq[:], scalar=EPS, in1=mv[:, 1:2],
                        op0=OP.add, op1=OP.add,
                    )
                    st2s.append(st2)
                gsp = ps_m.tile([G, 2], F32, name="gsp", tag="mm512")
                for ci in range(CT):
                    nc.tensor.matmul(
                        gsp[:], g8_t[:, ci, :], st2s[ci][:],
                        start=(ci == 0), stop=(ci == CT - 1),
                    )
                gss = small.tile([G, 2], F32, name="gss")
                nc.vector.tensor_copy(out=gss[:], in_=gsp[:])
                # v = (E[x^2]+eps) - mean^2 ; rstd = rsqrt(v)
                gsq = small.tile([G, 1], F32, name="gsq")
                nc.vector.tensor_mul(out=gsq[:], in0=gss[:, 0:1], in1=gss[:, 0:1])
                gv = small.tile([G, 1], F32, name="gv")
                nc.vector.scalar_tensor_tensor(
                    out=gv[:], in0=gsq[:], scalar=-1.0, in1=gss[:, 1:2],
                    op0=OP.mult, op1=OP.add,
                )
                rc = small.tile([G, 1], F32, name="rc")
                nc.vector.reciprocal(out=rc[:], in_=gv[:])
                r = small.tile([G, 1], F32, name="rn0")
                nc.vector.tensor_scalar_min(r[:], rc[:], 1.0)
                sg2 = small.tile([G, 2], F32R, name="sg2")
                nc.vector.tensor_copy(out=sg2[:, 0:1], in_=gss[:, 0:1])
                for it in range(3):
                    t1 = small.tile([G, 1], F32, name="nw_t1")
                    nc.vector.tensor_mul(out=t1[:], in0=r[:], in1=r[:])
                    t2 = small.tile([G, 1], F32, name="nw_t2")
                    nc.vector.scalar_tensor_tensor(
                        out=t2[:], in0=t1[:], scalar=-0.5, in1=gv[:],
                        op0=OP.mult, op1=OP.mult,
                    )
                    dst = sg2[:, 1:2] if it == 2 else small.tile(
                        [G, 1], F32, name="nw_r"
                    )
                    nc.vector.scalar_tensor_tensor(
                        out=dst, in0=t2[:], scalar=1.5, in1=r[:],
                        op0=OP.add, op1=OP.mult,
                    )
                    if it < 2:
                        r = dst
                s["sg2"] = sg2

            def gn_post(s):
                """Broadcast group stats to channels; per-channel affine
                A = rstd*gamma, B2 = mean*A - beta (h computed as x*A - B2)."""
                a_t = small.tile([128, CT], F32, name="a_vec")
                b2_t = small.tile([128, CT], F32, name="b2_vec")
                for ct in range(CT):
                    csp = ps_m.tile([128, 2], F32, name="csp", tag="mm512")
                    nc.tensor.matmul(
                        csp[:], gt_t[:, ct, :], s["sg2"][:], start=True, stop=True
                    )
                    nc.vector.tensor_mul(
                        out=a_t[:, ct : ct + 1], in0=csp[:, 1:2],
                        in1=vec_t[:, GAM, ct : ct + 1],
                    )
                    nc.vector.scalar_tensor_tensor(
                        out=b2_t[:, ct : ct + 1], in0=csp[:, 0:1],
                        scalar=a_t[:, ct : ct + 1], in1=vec_t[:, BET, ct : ct + 1],
                        op0=OP.mult, op1=OP.subtract,
                    )
                s["a"], s["b2"] = a_t, b2_t

            def stage_proj(s):
                """h = x*A - B2 ; q,k (natural) ; vT (transposed) projections."""
                ht = big.tile([128, CT, N], F32R, name="hT")
                for ct in range(CT):
                    nc.vector.tensor_scalar(
                        ht[:, ct, :], s["x"][:, ct, :],
                        s["a"][:, ct : ct + 1], s["b2"][:, ct : ct + 1],
                        OP.mult, OP.subtract,
                    )
                s["h"] = ht

                qt = big.tile([128, CT, N], QK_DT, name="qT")
                kt = big.tile([128, CT, N], QK_DT, name="kT")
                for dst, w_t, bias_idx in ((qt, wq_t, BQ), (kt, wk_t, BK)):
                    for co in range(CT):
                        acc = ps_a.tile([128, N], F32, name="acc", tag="acc")
                        for mch in range(MCH):
                            msl = slice(mch * 512, (mch + 1) * 512)
                            for ci in range(CT):
                                nc.tensor.matmul(
                                    acc[:, msl],
                                    w_t[:, ci, co * 128 : (co + 1) * 128],
                                    ht[:, ci, msl],
                                    start=(ci == 0),
                                    stop=(ci == CT - 1),
                                )
                        nc.scalar.activation(
                            out=dst[:, co, :], in_=acc[:], func=AF.Identity,
                            bias=vec_t[:, bias_idx, co : co + 1], scale=1.0,
                        )
                s["q"], s["k"] = qt, kt

                vts = []
                for nt in range(NT):
                    vp = ps_m.tile([128, C], F32, name="vp", tag="mm512")
                    for ci in range(CT):
                        nc.tensor.matmul(
                            vp[:],
                            ht[:, ci, nt * 128 : (nt + 1) * 128],
                            wv_t[:, ci, :],
                            start=(ci == 0),
                            stop=(ci == CT - 1),
                        )
                    vt = vtp.tile([128, C], PV_DT, name=f"vt{nt}")
                    nc.vector.tensor_add(out=vt[:], in0=vp[:], in1=bvb_t[:])
                    vts.append(vt)
                s["v"] = vts

            def stage_b(s, nxt):
                """scores^T -> exp -> pT ; row sums; next batch's gn chain
                interleaved so its serial DVE latency hides under PE work."""
                rs = ps_rs.tile([128, N], F32, name="rsp")
                pts = []
                for nt in range(NT):
                    stp = ps_a.tile([128, N], F32, name="stp", tag="acc")
                    for mch in range(MCH):
                        msl = slice(mch * 512, (mch + 1) * 512)
                        for ci in range(CT):
                            nc.tensor.matmul(
                                stp[:, msl],
                                s["k"][:, ci, nt * 128 : (nt + 1) * 128],
                                s["q"][:, ci, msl],
                                start=(ci == 0),
                                stop=(ci == CT - 1),
                            )
                    pt = ptp.tile([128, N], PV_DT, name=f"pt{nt}")
                    nc.scalar.activation(
                        out=pt[:], in_=stp[:], func=AF.Exp, bias=0.0, scale=SCALE
                    )
                    pts.append(pt)
                    for mch in range(MCH):
                        msl = slice(mch * 512, (mch + 1) * 512)
                        nc.tensor.matmul(
                            rs[:, msl], ones_t[:], pt[:, msl],
                            start=(nt == 0), stop=(nt == NT - 1),
                        )
                    if nt == 5 and nxt is not None:
                        gn_pre(nxt)
                s["p"] = pts
                s["rs"] = rs
                if nxt is not None:
                    gn_post(nxt)

            def stage_c(s):
                """1/rowsum; attend (+normalize); project (+bias+residual)."""
                scr = misc.tile([128, N], F32, name="scr")
                rcp = misc.tile([128, N], F32, name="rcp")
                nc.vector.reciprocal_approx_accurate(
                    out=rcp[:], in_=s["rs"][:], scratch=scr[:]
                )

                ont = big.tile([128, CT, N], F32R, name="onT")
                for ct in range(CT):
                    for mch in range(MCH):
                        msl = slice(mch * 512, (mch + 1) * 512)
                        ap_ = ps_m.tile([128, 512], F32, name="attp", tag="mm512")
                        for nt in range(NT):
                            nc.tensor.matmul(
                                ap_[:],
                                s["v"][nt][:, ct * 128 : (ct + 1) * 128],
                                s["p"][nt][:, msl],
                                start=(nt == 0),
                                stop=(nt == NT - 1),
                            )
                        nc.vector.tensor_mul(
                            out=ont[:, ct, msl], in0=ap_[:], in1=rcp[:, msl]
                        )

                outf = big.tile([128, CT, N], F32, name="outf")
                for co in range(CT):
                    for mch in range(MCH):
                        msl = slice(mch * 512, (mch + 1) * 512)
                        pp = ps_m.tile([128, 512], F32, name="pp", tag="mm512")
                        for ci in range(CT):
                            nc.tensor.matmul(
                                pp[:],
                                wp_t[:, ci, co * 128 : (co + 1) * 128],
                                ont[:, ci, msl],
                                start=(ci == 0),
                                stop=(ci == CT - 1),
                            )
                        nc.vector.scalar_tensor_tensor(
                            out=outf[:, co, msl],
                            in0=pp[:],
                            scalar=vec_t[:, BP, co : co + 1],
                            in1=s["x"][:, co, msl],
                            op0=OP.add,
                            op1=OP.add,
                        )
                nc.sync.dma_start(
                    out=out_d[s["b"]].rearrange("(ct p) n -> p ct n", p=128),
                    in_=outf[:],
                )

            # ---------------- emission schedule ----------------
            gn_pre(states[0])
            gn_post(states[0])
            stage_proj(states[0])
            for b in range(B_LOC):
                nxt = None
                if b + 1 < B_LOC:
                    states[b + 1] = load(b + 1)
                    nxt = states[b + 1]
                stage_b(states[b], nxt)
                stage_c(states[b])
                states[b] = None
                if nxt is not None:
                    stage_proj(nxt)

    nc.finalize()
    return nc


_NC = None


def _get_nc():
    global _NC
    if _NC is None:
        _NC = _build_nc()
    return _NC


def _make_in_maps(inputs):
    x = np.asarray(inputs["x"], dtype=np.float32).reshape(B, C, N)
    g8 = np.zeros((C, G), np.float32)
    for c in range(C):
        g8[c, c // 8] = 0.125
    gt = np.zeros((G, C), np.float32)
    for c in range(C):
        gt[c // 8, c] = 1.0
    vecs = np.stack(
        [
            np.asarray(inputs["gamma"], np.float32),
            np.asarray(inputs["beta"], np.float32),
            np.asarray(inputs["bq"], np.float32),
            np.asarray(inputs["bk"], np.float32),
            np.asarray(inputs["bp"], np.float32),
        ]
    )

    shared = {
        "wqT": np.ascontiguousarray(np.asarray(inputs["wq"], np.float32).T),
        "wkT": np.ascontiguousarray(np.asarray(inputs["wk"], np.float32).T),
        "wvT": np.ascontiguousarray(np.asarray(inputs["wv"], np.float32).T),
        "wpT": np.ascontiguousarray(np.asarray(inputs["wp"], np.float32).T),
        "vecs": vecs,
        "bv": np.asarray(inputs["bv"], np.float32),
        "g8": g8,
        "gt": gt,
        "ones": np.ones((128,), ml_dtypes.bfloat16),
    }
    in_maps = []
    for i in range(N_CORES):
        m = dict(shared)
        m["x"] = np.ascontiguousarray(x[i * B_LOC : (i + 1) * B_LOC])
        in_maps.append(m)
    return in_maps


def _run(inputs, trace=False):
    from concourse.bass_utils import run_bass_kernel_spmd

    nc = _get_nc()
    in_maps = _make_in_maps(inputs)
    res = run_bass_kernel_spmd(
        nc, in_maps, core_ids=list(range(N_CORES)), trace=trace
    )
    out = np.concatenate([r["out"] for r in res.results], axis=0)
    return out.reshape(B, C, 32, 32).astype(np.float32), res


def kernel(**inputs) -> np.ndarray:
    out, _ = _run(inputs, trace=False)
    return out


# revision 23
# speedup vs baseline: 1.1975x; 1.0035x over previous
"""AttnBlock (GroupNorm + single-head self-attention + residual) on 8 Trainium2
NeuronCores, pure data-parallel over the batch dimension.

Reference math (per batch b):
    h = GroupNorm32(x) * gamma + beta               # [C, N], C=256, N=1024
    q = wq @ h + bq ; k = wk @ h + bk ; v = wv @ h + bv
    s[m, n] = <q[:, m], k[:, n]> / sqrt(C)
    w = softmax(s, axis=n)
    o[c, m] = sum_n w[m, n] v[c, n]
    out = x + wp @ o + bp

Device-side strategy (per core: 4 batches):
  - q, k in natural [c, n] layout; scores computed TRANSPOSED
    (sT[n, m] = k^T q) so exp(sT) is already partition-major in n — the
    contraction axis of the attend matmul — avoiding any 128x128 transposes.
  - Softmax runs without max-subtraction (scores are ~N(0,1); exp is safe in
    fp32) so exp comes straight off the scores PSUM.
  - Row sums via a ones[128,128] stationary matmul, which also broadcasts the
    denominators to all partitions for free. 1/x via the custom-DVE
    reciprocal_approx_accurate; GroupNorm rstd via a DVE Newton rsqrt. ScalarE
    then only ever runs Exp/Identity (one table set, no table-switch stalls).
  - v is computed directly transposed (vT = h^T @ wvT); softmax normalization
    is folded into the attend PSUM eviction; proj bias + residual are folded
    into the final eviction (scalar_tensor_tensor).
  - All big matmuls run as float32r (full PE rate at free-dim >= 256), fp32
    storage and PSUM accumulation everywhere.
  - Emission interleaves batches: batch b+1's GroupNorm stat chain (serial
    small DVE ops) hides under batch b's scores/exp phase; batch b+1's
    h/q/k/vT projections fill the PE while batch b's exp tail finishes.
"""

import sys

sys.path.insert(0, "/opt/trn_rl_repo")

import ml_dtypes
import numpy as np

import concourse.bass as bass
import concourse.tile as tile
from concourse import bacc, mybir

F32 = mybir.dt.float32
F32R = mybir.dt.float32r
BF16 = mybir.dt.bfloat16

# bf16 measured NO faster on PE (1 elem/cycle streaming regardless of dtype),
# so the whole attention path stays fp32r for accuracy.
PV_DT = F32R
QK_DT = F32R
AF = mybir.ActivationFunctionType
OP = mybir.AluOpType

N_CORES = 8
B = 32  # full batch
B_LOC = B // N_CORES  # batches per core
C = 256
CT = 2  # channel tiles of 128
N = 1024  # spatial (32*32)
NT = 8  # spatial partition-tiles of 128
MCH = 2  # spatial free-dim chunks of 512
G = 32  # groups
EPS = 1e-5
SCALE = C ** -0.5  # 1/16


def _bcast_ap(handle, nparts):
    """Partition-broadcast read AP for a 1-D DRAM tensor."""
    ap = handle[:]
    return bass.AP(tensor=ap.tensor, offset=ap.offset, ap=[[0, nparts]] + list(ap.ap))


def _build_nc(qk_bias=False):
    nc = bacc.Bacc()

    x_d = nc.declare_dram_parameter("x", [B_LOC, C, N], F32, isOutput=False)
    if qk_bias:
        wq_d = nc.declare_dram_parameter("wqT", [C, C], F32, isOutput=False)
        wk_d = nc.declare_dram_parameter("wkT", [C, C], F32, isOutput=False)
    else:
        # wa = wk.T @ wq (host-folded): scores = h^T (wa^T) h needs one
        # projection u = wa^T... stored so lhsT layout matches other weights
        wa_d = nc.declare_dram_parameter("waT", [C, C], F32, isOutput=False)
    wv_d = nc.declare_dram_parameter("wvT", [C, C], F32, isOutput=False)
    wp_d = nc.declare_dram_parameter("wpT", [C, C], F32, isOutput=False)
    vec_d = nc.declare_dram_parameter("vecs", [5, C], F32, isOutput=False)
    bv_d = nc.declare_dram_parameter("bv", [C], F32, isOutput=False)
    ones_d = nc.declare_dram_parameter("ones", [128], F32, isOutput=False)
    g8_d = nc.declare_dram_parameter("g8", [C, G], F32, isOutput=False)
    gt_d = nc.declare_dram_parameter("gt", [G, C], F32, isOutput=False)
    out_d = nc.declare_dram_parameter("out", [B_LOC, C, N], F32, isOutput=True)

    with tile.TileContext(nc) as tc:
        with (
            tc.tile_pool(name="consts", bufs=1) as consts,
            tc.tile_pool(name="big", bufs=2) as big,
            tc.tile_pool(name="vtp", bufs=2) as vtp,
            tc.tile_pool(name="ptp", bufs=2) as ptp,
            tc.tile_pool(name="misc", bufs=2) as misc,
            tc.tile_pool(name="small", bufs=3) as small,
            tc.tile_pool(name="ps_a", bufs=2, space="PSUM") as ps_a,
            tc.tile_pool(name="ps_rs", bufs=1, space="PSUM") as ps_rs,
            tc.tile_pool(name="ps_m", bufs=2, space="PSUM") as ps_m,
        ):
            # ------- batch-0 input load first: nothing queues ahead of it
            def load(b):
                s = {"b": b}
                xt = big.tile([128, CT, N], F32, name="xT")
                # chunked so bn_stats of a subgroup can start before the
                # whole batch input has landed
                for ct in range(CT):
                    for half in range(2):
                        hsl = slice(half * 512, (half + 1) * 512)
                        nc.sync.dma_start(
                            out=xt[:, ct, hsl],
                            in_=x_d[b, ct * 128 : (ct + 1) * 128, hsl],
                        )
                s["x"] = xt
                return s

            states = [None] * B_LOC
            states[0] = load(0)

            # ------- constants / weights (one DMA each, gpsimd queue)
            w_tiles = {}
            wlist = (
                (("wq", wq_d), ("wk", wk_d)) if qk_bias else (("wa", wa_d),)
            ) + (("wv", wv_d), ("wp", wp_d))
            for nm, d in wlist:
                t = consts.tile([128, CT, C], F32R, name=f"{nm}_t")
                nc.gpsimd.dma_start(
                    out=t[:],
                    in_=d[:, :].rearrange("(ci p) o -> p ci o", p=128).bitcast(F32R),
                )
                w_tiles[nm] = t
            wv_t, wp_t = w_tiles["wv"], w_tiles["wp"]

            # gamma, beta, bq, bk, bp as per-partition [128, 5, CT]
            vec_t = consts.tile([128, 5, CT], F32, name="vec_t")
            nc.gpsimd.dma_start(
                out=vec_t[:], in_=vec_d[:, :].rearrange("v (ct p) -> p v ct", p=128)
            )
            GAM, BET, BQ, BK, BP = range(5)

            g8_t = consts.tile([128, CT, G], F32R, name="g8_t")
            nc.gpsimd.dma_start(
                out=g8_t[:],
                in_=g8_d[:, :].rearrange("(ci p) g -> p ci g", p=128).bitcast(F32R),
            )
            gt_t = consts.tile([G, CT, 128], F32R, name="gt_t")
            nc.gpsimd.dma_start(
                out=gt_t[:],
                in_=gt_d[:, :].rearrange("g (ct p) -> g ct p", p=128).bitcast(F32R),
            )
            ones_t = consts.tile([128, 128], F32R, name="ones_t")
            nc.gpsimd.dma_start(
                out=ones_t[:], in_=_bcast_ap(ones_d, 128).bitcast(F32R)
            )
            bvb_t = consts.tile([128, C], F32, name="bvb_t")
            nc.gpsimd.dma_start(out=bvb_t[:], in_=_bcast_ap(bv_d, 128))

            # ---------------- per-batch stages ----------------

            def gn_pre(s):
                """bn stats -> per-channel [mean, E[x^2]+eps] -> group stats
                -> Newton rsqrt -> sg2 = [mean_g, rstd_g]. Mostly small serial
                DVE work; emitted where PE is busy with the previous batch."""
                xt = s["x"]
                st2s = []
                for ct in range(CT):
                    xin = xt[:, ct, :].rearrange("p (s f) -> p s f", f=512)
                    st6 = small.tile([128, 2, 6], F32, name="st6")
                    for sg in range(2):
                        nc.vector.bn_stats(out=st6[:, sg, :], in_=xin[:, sg, :])
                    mv = small.tile([128, 2], F32, name="mv")
                    nc.vector.bn_aggr(out=mv[:], in_=st6[:])
                    st2 = small.tile([128, 2], F32R, name=f"st2_{ct}")
                    nc.vector.tensor_copy(out=st2[:, 0:1], in_=mv[:, 0:1])
                    sq = small.tile([128, 1], F32, name="sq")
                    nc.vector.tensor_mul(out=sq[:], in0=mv[:, 0:1], in1=mv[:, 0:1])
                    # col1 = E[x^2] + eps  (G8 rows sum to 1, so eps survives)
                    nc.vector.scalar_tensor_tensor(
                        out=st2[:, 1:2], in0=sq[:], scalar=EPS, in1=mv[:, 1:2],
                        op0=OP.add, op1=OP.add,
                    )
                    st2s.append(st2)
                gsp = ps_m.tile([G, 2], F32, name="gsp", tag="mm512")
                for ci in range(CT):
                    nc.tensor.matmul(
                        gsp[:], g8_t[:, ci, :], st2s[ci][:],
                        start=(ci == 0), stop=(ci == CT - 1),
                    )
                gss = small.tile([G, 2], F32, name="gss")
                nc.vector.tensor_copy(out=gss[:], in_=gsp[:])
                # v = (E[x^2]+eps) - mean^2 ; rstd = rsqrt(v)
                gsq = small.tile([G, 1], F32, name="gsq")
                nc.vector.tensor_mul(out=gsq[:], in0=gss[:, 0:1], in1=gss[:, 0:1])
                gv = small.tile([G, 1], F32, name="gv")
                nc.vector.scalar_tensor_tensor(
                    out=gv[:], in0=gsq[:], scalar=-1.0, in1=gss[:, 1:2],
                    op0=OP.mult, op1=OP.add,
                )
                rc = small.tile([G, 1], F32, name="rc")
                nc.vector.reciprocal(out=rc[:], in_=gv[:])
                r = small.tile([G, 1], F32, name="rn0")
                nc.vector.tensor_scalar_min(r[:], rc[:], 1.0)
                sg2 = small.tile([G, 2], F32R, name="sg2")
                nc.vector.tensor_copy(out=sg2[:, 0:1], in_=gss[:, 0:1])
                for it in range(3):
                    t1 = small.tile([G, 1], F32, name="nw_t1")
                    nc.vector.tensor_mul(out=t1[:], in0=r[:], in1=r[:])
                    t2 = small.tile([G, 1], F32, name="nw_t2")
                    nc.vector.scalar_tensor_tensor(
                        out=t2[:], in0=t1[:], scalar=-0.5, in1=gv[:],
                        op0=OP.mult, op1=OP.mult,
                    )
                    dst = sg2[:, 1:2] if it == 2 else small.tile(
                        [G, 1], F32, name="nw_r"
                    )
                    nc.vector.scalar_tensor_tensor(
                        out=dst, in0=t2[:], scalar=1.5, in1=r[:],
                        op0=OP.add, op1=OP.mult,
                    )
                    if it < 2:
                        r = dst
                s["sg2"] = sg2

            def gn_post(s):
                """Broadcast group stats to channels; per-channel affine
                A = rstd*gamma, B2 = mean*A - beta (h computed as x*A - B2)."""
                a_t = small.tile([128, CT], F32, name="a_vec")
                b2_t = small.tile([128, CT], F32, name="b2_vec")
                for ct in range(CT):
                    csp = ps_m.tile([128, 2], F32, name="csp", tag="mm512")
                    nc.tensor.matmul(
                        csp[:], gt_t[:, ct, :], s["sg2"][:], start=True, stop=True
                    )
                    nc.vector.tensor_mul(
                        out=a_t[:, ct : ct + 1], in0=csp[:, 1:2],
                        in1=vec_t[:, GAM, ct : ct + 1],
                    )
                    nc.vector.scalar_tensor_tensor(
                        out=b2_t[:, ct : ct + 1], in0=csp[:, 0:1],
                        scalar=a_t[:, ct : ct + 1], in1=vec_t[:, BET, ct : ct + 1],
                        op0=OP.mult, op1=OP.subtract,
                    )
                s["a"], s["b2"] = a_t, b2_t

            def stage_proj(s):
                """h = x*A - B2 ; q,k (natural) ; vT (transposed) projections."""
                ht = big.tile([128, CT, N], F32R, name="hT")
                for ct in range(CT):
                    nc.vector.tensor_scalar(
                        ht[:, ct, :], s["x"][:, ct, :],
                        s["a"][:, ct : ct + 1], s["b2"][:, ct : ct + 1],
                        OP.mult, OP.subtract,
                    )
                s["h"] = ht

                if qk_bias:
                    qt = big.tile([128, CT, N], QK_DT, name="qT")
                    kt = big.tile([128, CT, N], QK_DT, name="kT")
                    pairs = ((qt, w_tiles["wq"], BQ), (kt, w_tiles["wk"], BK))
                else:
                    # u = wa^T... : s[m,n] = sum_c h[c,m] u[c,n]
                    ut = big.tile([128, CT, N], QK_DT, name="qT")
                    pairs = ((ut, w_tiles["wa"], None),)
                for dst, w_t, bias_idx in pairs:
                    for co in range(CT):
                        acc = ps_a.tile([128, N], F32, name="acc", tag="acc")
                        for mch in range(MCH):
                            msl = slice(mch * 512, (mch + 1) * 512)
                            for ci in range(CT):
                                nc.tensor.matmul(
                                    acc[:, msl],
                                    w_t[:, ci, co * 128 : (co + 1) * 128],
                                    ht[:, ci, msl],
                                    start=(ci == 0),
                                    stop=(ci == CT - 1),
                                )
                        nc.scalar.activation(
                            out=dst[:, co, :], in_=acc[:], func=AF.Identity,
                            bias=(0.0 if bias_idx is None
                                  else vec_t[:, bias_idx, co : co + 1]),
                            scale=1.0,
                        )
                if qk_bias:
                    s["q"], s["k"] = qt, kt
                else:
                    # sT[n,m] = sum_c u[c,n] h[c,m]: u is stationary, h moving
                    s["q"], s["k"] = ht, ut

                vts = []
                for nt in range(NT):
                    vp = ps_m.tile([128, C], F32, name="vp", tag="mm512")
                    for ci in range(CT):
                        nc.tensor.matmul(
                            vp[:],
                            ht[:, ci, nt * 128 : (nt + 1) * 128],
                            wv_t[:, ci, :],
                            start=(ci == 0),
                            stop=(ci == CT - 1),
                        )
                    vt = vtp.tile([128, C], PV_DT, name=f"vt{nt}")
                    nc.vector.tensor_add(out=vt[:], in0=vp[:], in1=bvb_t[:])
                    vts.append(vt)
                s["v"] = vts

            def stage_b(s, nxt):
                """scores^T -> exp -> pT ; row sums; next batch's gn chain
                interleaved so its serial DVE latency hides under PE work."""
                rs = ps_rs.tile([128, N], F32, name="rsp")
                pts = []
                for nt in range(NT):
                    stp = ps_a.tile([128, N], F32, name="stp", tag="acc")
                    for mch in range(MCH):
                        msl = slice(mch * 512, (mch + 1) * 512)
                        for ci in range(CT):
                            nc.tensor.matmul(
                                stp[:, msl],
                                s["k"][:, ci, nt * 128 : (nt + 1) * 128],
                                s["q"][:, ci, msl],
                                start=(ci == 0),
                                stop=(ci == CT - 1),
                            )
                    pt = ptp.tile([128, N], PV_DT, name=f"pt{nt}")
                    nc.scalar.activation(
                        out=pt[:], in_=stp[:], func=AF.Exp, bias=0.0, scale=SCALE
                    )
                    pts.append(pt)
                    for mch in range(MCH):
                        msl = slice(mch * 512, (mch + 1) * 512)
                        nc.tensor.matmul(
                            rs[:, msl], ones_t[:], pt[:, msl],
                            start=(nt == 0), stop=(nt == NT - 1),
                        )
                    if nt == 5 and nxt is not None:
                        gn_pre(nxt)
                s["p"] = pts
                s["rs"] = rs
                if nxt is not None:
                    gn_post(nxt)

            def stage_c(s):
                """1/rowsum; attend (+normalize); project (+bias+residual)."""
                scr = misc.tile([128, N], F32, name="scr")
                rcp = misc.tile([128, N], F32, name="rcp")
                nc.vector.reciprocal_approx_accurate(
                    out=rcp[:], in_=s["rs"][:], scratch=scr[:]
                )

                ont = big.tile([128, CT, N], F32R, name="onT")
                for ct in range(CT):
                    for mch in range(MCH):
                        msl = slice(mch * 512, (mch + 1) * 512)
                        ap_ = ps_m.tile([128, 512], F32, name="attp", tag="mm512")
                        for nt in range(NT):
                            nc.tensor.matmul(
                                ap_[:],
                                s["v"][nt][:, ct * 128 : (ct + 1) * 128],
                                s["p"][nt][:, msl],
                                start=(nt == 0),
                                stop=(nt == NT - 1),
                            )
                        nc.vector.tensor_mul(
                            out=ont[:, ct, msl], in0=ap_[:], in1=rcp[:, msl]
                        )

                outf = big.tile([128, CT, N], F32, name="outf")
                for co in range(CT):
                    for mch in range(MCH):
                        msl = slice(mch * 512, (mch + 1) * 512)
                        pp = ps_m.tile([128, 512], F32, name="pp", tag="mm512")
                        for ci in range(CT):
                            nc.tensor.matmul(
                                pp[:],
                                wp_t[:, ci, co * 128 : (co + 1) * 128],
                                ont[:, ci, msl],
                                start=(ci == 0),
                                stop=(ci == CT - 1),
                            )
                        nc.vector.scalar_tensor_tensor(
                            out=outf[:, co, msl],
                            in0=pp[:],
                            scalar=vec_t[:, BP, co : co + 1],
                            in1=s["x"][:, co, msl],
                            op0=OP.add,
                            op1=OP.add,
                        )
                nc.sync.dma_start(
                    out=out_d[s["b"]].rearrange("(ct p) n -> p ct n", p=128),
                    in_=outf[:],
                )

            # ---------------- emission schedule ----------------
            gn_pre(states[0])
            gn_post(states[0])
            stage_proj(states[0])
            for b in range(B_LOC):
                nxt = None
                if b + 1 < B_LOC:
                    states[b + 1] = load(b + 1)
                    nxt = states[b + 1]
                stage_b(states[b], nxt)
                stage_c(states[b])
                states[b] = None
                if nxt is not None:
                    stage_proj(nxt)

    nc.finalize()
    return nc


_NC = {}


def _get_nc(qk_bias):
    if qk_bias not in _NC:
        _NC[qk_bias] = _build_nc(qk_bias=qk_bias)
    return _NC[qk_bias]


def _make_in_maps(inputs, qk_bias):
    x = np.asarray(inputs["x"], dtype=np.float32).reshape(B, C, N)
    g8 = np.zeros((C, G), np.float32)
    for c in range(C):
        g8[c, c // 8] = 0.125
    gt = np.zeros((G, C), np.float32)
    for c in range(C):
        gt[c // 8, c] = 1.0
    vecs = np.stack(
        [
            np.asarray(inputs["gamma"], np.float32),
            np.asarray(inputs["beta"], np.float32),
            np.asarray(inputs["bq"], np.float32),
            np.asarray(inputs["bk"], np.float32),
            np.asarray(inputs["bp"], np.float32),
        ]
    )

    shared = {
        "wvT": np.ascontiguousarray(np.asarray(inputs["wv"], np.float32).T),
        "wpT": np.ascontiguousarray(np.asarray(inputs["wp"], np.float32).T),
        "vecs": vecs,
        "bv": np.asarray(inputs["bv"], np.float32),
        "g8": g8,
        "gt": gt,
        "ones": np.ones((128,), np.float32),
    }
    if qk_bias:
        shared["wqT"] = np.ascontiguousarray(np.asarray(inputs["wq"], np.float32).T)
        shared["wkT"] = np.ascontiguousarray(np.asarray(inputs["wk"], np.float32).T)
    else:
        wa = np.asarray(inputs["wk"], np.float64).T @ np.asarray(
            inputs["wq"], np.float64
        )
        shared["waT"] = np.ascontiguousarray(wa.astype(np.float32))
    in_maps = []
    for i in range(N_CORES):
        m = dict(shared)
        m["x"] = np.ascontiguousarray(x[i * B_LOC : (i + 1) * B_LOC])
        in_maps.append(m)
    return in_maps


def _run(inputs, trace=False):
    from concourse.bass_utils import run_bass_kernel_spmd

    qk_bias = bool(
        np.any(np.asarray(inputs["bq"])) or np.any(np.asarray(inputs["bk"]))
    )
    nc = _get_nc(qk_bias)
    in_maps = _make_in_maps(inputs, qk_bias)
    res = run_bass_kernel_spmd(
        nc, in_maps, core_ids=list(range(N_CORES)), trace=trace
    )
    out = np.concatenate([r["out"] for r in res.results], axis=0)
    return out.reshape(B, C, 32, 32).astype(np.float32), res


def kernel(**inputs) -> np.ndarray:
    out, _ = _run(inputs, trace=False)
    return out


# revision 24
# speedup vs baseline: 1.2350x; 1.0313x over previous
"""AttnBlock (GroupNorm + single-head self-attention + residual) on 8 Trainium2
NeuronCores, pure data-parallel over the batch dimension.

Reference math (per batch b):
    h = GroupNorm32(x) * gamma + beta               # [C, N], C=256, N=1024
    q = wq @ h + bq ; k = wk @ h + bk ; v = wv @ h + bv
    s[m, n] = <q[:, m], k[:, n]> / sqrt(C)
    w = softmax(s, axis=n)
    o[c, m] = sum_n w[m, n] v[c, n]
    out = x + wp @ o + bp

Device-side strategy (per core: 4 batches):
  - q, k in natural [c, n] layout; scores computed TRANSPOSED
    (sT[n, m] = k^T q) so exp(sT) is already partition-major in n — the
    contraction axis of the attend matmul — avoiding any 128x128 transposes.
  - Softmax runs without max-subtraction (scores are ~N(0,1); exp is safe in
    fp32) so exp comes straight off the scores PSUM.
  - Row sums via a ones[128,128] stationary matmul, which also broadcasts the
    denominators to all partitions for free. 1/x via the custom-DVE
    reciprocal_approx_accurate; GroupNorm rstd via a DVE Newton rsqrt. ScalarE
    then only ever runs Exp/Identity (one table set, no table-switch stalls).
  - v is computed directly transposed (vT = h^T @ wvT); softmax normalization
    is folded into the attend PSUM eviction; proj bias + residual are folded
    into the final eviction (scalar_tensor_tensor).
  - All big matmuls run as float32r (full PE rate at free-dim >= 256), fp32
    storage and PSUM accumulation everywhere.
  - Emission interleaves batches: batch b+1's GroupNorm stat chain (serial
    small DVE ops) hides under batch b's scores/exp phase; batch b+1's
    h/q/k/vT projections fill the PE while batch b's exp tail finishes.
"""

import sys

sys.path.insert(0, "/opt/trn_rl_repo")

import ml_dtypes
import numpy as np

import concourse.bass as bass
import concourse.tile as tile
from concourse import bacc, mybir

F32 = mybir.dt.float32
F32R = mybir.dt.float32r
BF16 = mybir.dt.bfloat16

# bf16 measured NO faster on PE (1 elem/cycle streaming regardless of dtype),
# so the whole attention path stays fp32r for accuracy.
PV_DT = F32R
QK_DT = F32R
AF = mybir.ActivationFunctionType
OP = mybir.AluOpType

N_CORES = 8
B = 32  # full batch
B_LOC = B // N_CORES  # batches per core
C = 256
CT = 2  # channel tiles of 128
N = 1024  # spatial (32*32)
NT = 8  # spatial partition-tiles of 128
MCH = 2  # spatial free-dim chunks of 512
G = 32  # groups
EPS = 1e-5
SCALE = C ** -0.5  # 1/16


def _bcast_ap(handle, nparts):
    """Partition-broadcast read AP for a 1-D DRAM tensor."""
    ap = handle[:]
    return bass.AP(tensor=ap.tensor, offset=ap.offset, ap=[[0, nparts]] + list(ap.ap))


def _build_nc(qk_bias=False):
    nc = bacc.Bacc()

    x_d = nc.declare_dram_parameter("x", [B_LOC, C, N], F32, isOutput=False)
    if qk_bias:
        wq_d = nc.declare_dram_parameter("wqT", [C, C], F32, isOutput=False)
        wk_d = nc.declare_dram_parameter("wkT", [C, C], F32, isOutput=False)
    else:
        # wa = wk.T @ wq (host-folded): scores = h^T (wa^T) h needs one
        # projection u = wa^T... stored so lhsT layout matches other weights
        wa_d = nc.declare_dram_parameter("waT", [C, C], F32, isOutput=False)
    wv_d = nc.declare_dram_parameter("wvT", [C, C], F32, isOutput=False)
    wp_d = nc.declare_dram_parameter("wpT", [C, C], F32, isOutput=False)
    vec_d = nc.declare_dram_parameter("vecs", [5, C], F32, isOutput=False)
    bv_d = nc.declare_dram_parameter("bv", [C], F32, isOutput=False)
    ones_d = nc.declare_dram_parameter("ones", [128], F32, isOutput=False)
    g8_d = nc.declare_dram_parameter("g8", [C, G], F32, isOutput=False)
    gt_d = nc.declare_dram_parameter("gt", [G, C], F32, isOutput=False)
    out_d = nc.declare_dram_parameter("out", [B_LOC, C, N], F32, isOutput=True)

    with tile.TileContext(nc) as tc:
        with (
            tc.tile_pool(name="consts", bufs=1) as consts,
            tc.tile_pool(name="big", bufs=2) as big,
            tc.tile_pool(name="vtp", bufs=2) as vtp,
            tc.tile_pool(name="ptp", bufs=2) as ptp,
            tc.tile_pool(name="misc", bufs=2) as misc,
            tc.tile_pool(name="small", bufs=3) as small,
            tc.tile_pool(name="ps_a", bufs=2, space="PSUM") as ps_a,
            tc.tile_pool(name="ps_rs", bufs=1, space="PSUM") as ps_rs,
            tc.tile_pool(name="ps_m", bufs=2, space="PSUM") as ps_m,
        ):
            # ------- batch-0 input load first: nothing queues ahead of it
            def load(b):
                s = {"b": b}
                xt = big.tile([128, CT, N], F32, name="xT")
                # split per channel-tile so ct0's bn_stats can start while
                # ct1 is still landing
                for ct in range(CT):
                    nc.sync.dma_start(
                        out=xt[:, ct, :],
                        in_=x_d[b, ct * 128 : (ct + 1) * 128, :],
                    )
                s["x"] = xt
                return s

            states = [None] * B_LOC
            states[0] = load(0)

            # ------- constants / weights (one DMA each, gpsimd queue)
            w_tiles = {}
            wlist = (
                (("wq", wq_d), ("wk", wk_d)) if qk_bias else (("wa", wa_d),)
            ) + (("wv", wv_d), ("wp", wp_d))
            for nm, d in wlist:
                t = consts.tile([128, CT, C], F32R, name=f"{nm}_t")
                nc.gpsimd.dma_start(
                    out=t[:],
                    in_=d[:, :].rearrange("(ci p) o -> p ci o", p=128).bitcast(F32R),
                )
                w_tiles[nm] = t
            wv_t, wp_t = w_tiles["wv"], w_tiles["wp"]

            # gamma, beta, bq, bk, bp as per-partition [128, 5, CT]
            vec_t = consts.tile([128, 5, CT], F32, name="vec_t")
            nc.gpsimd.dma_start(
                out=vec_t[:], in_=vec_d[:, :].rearrange("v (ct p) -> p v ct", p=128)
            )
            GAM, BET, BQ, BK, BP = range(5)

            g8_t = consts.tile([128, CT, G], F32R, name="g8_t")
            nc.gpsimd.dma_start(
                out=g8_t[:],
                in_=g8_d[:, :].rearrange("(ci p) g -> p ci g", p=128).bitcast(F32R),
            )
            gt_t = consts.tile([G, CT, 128], F32R, name="gt_t")
            nc.gpsimd.dma_start(
                out=gt_t[:],
                in_=gt_d[:, :].rearrange("g (ct p) -> g ct p", p=128).bitcast(F32R),
            )
            ones_t = consts.tile([128, 128], F32R, name="ones_t")
            nc.gpsimd.dma_start(
                out=ones_t[:], in_=_bcast_ap(ones_d, 128).bitcast(F32R)
            )
            bvb_t = consts.tile([128, C], F32, name="bvb_t")
            nc.gpsimd.dma_start(out=bvb_t[:], in_=_bcast_ap(bv_d, 128))

            # ---------------- per-batch stages ----------------

            def gn_pre(s):
                """bn stats -> per-channel [mean, E[x^2]+eps] -> group stats
                -> Newton rsqrt -> sg2 = [mean_g, rstd_g]. Mostly small serial
                DVE work; emitted where PE is busy with the previous batch."""
                xt = s["x"]
                st2s = []
                for ct in range(CT):
                    xin = xt[:, ct, :].rearrange("p (s f) -> p s f", f=512)
                    st6 = small.tile([128, 2, 6], F32, name="st6")
                    for sg in range(2):
                        nc.vector.bn_stats(out=st6[:, sg, :], in_=xin[:, sg, :])
                    mv = small.tile([128, 2], F32, name="mv")
                    nc.vector.bn_aggr(out=mv[:], in_=st6[:])
                    st2 = small.tile([128, 2], F32R, name=f"st2_{ct}")
                    nc.vector.tensor_copy(out=st2[:, 0:1], in_=mv[:, 0:1])
                    sq = small.tile([128, 1], F32, name="sq")
                    nc.vector.tensor_mul(out=sq[:], in0=mv[:, 0:1], in1=mv[:, 0:1])
                    # col1 = E[x^2] + eps  (G8 rows sum to 1, so eps survives)
                    nc.vector.scalar_tensor_tensor(
                        out=st2[:, 1:2], in0=sq[:], scalar=EPS, in1=mv[:, 1:2],
                        op0=OP.add, op1=OP.add,
                    )
                    st2s.append(st2)
                gsp = ps_m.tile([G, 2], F32, name="gsp", tag="mm512")
                for ci in range(CT):
                    nc.tensor.matmul(
                        gsp[:], g8_t[:, ci, :], st2s[ci][:],
                        start=(ci == 0), stop=(ci == CT - 1),
                    )
                gss = small.tile([G, 2], F32, name="gss")
                nc.vector.tensor_copy(out=gss[:], in_=gsp[:])
                # v = (E[x^2]+eps) - mean^2 ; rstd = rsqrt(v)
                gsq = small.tile([G, 1], F32, name="gsq")
                nc.vector.tensor_mul(out=gsq[:], in0=gss[:, 0:1], in1=gss[:, 0:1])
                gv = small.tile([G, 1], F32, name="gv")
                nc.vector.scalar_tensor_tensor(
                    out=gv[:], in0=gsq[:], scalar=-1.0, in1=gss[:, 1:2],
                    op0=OP.mult, op1=OP.add,
                )
                rc = small.tile([G, 1], F32, name="rc")
                nc.vector.reciprocal(out=rc[:], in_=gv[:])
                r = small.tile([G, 1], F32, name="rn0")
                nc.vector.tensor_scalar_min(r[:], rc[:], 1.0)
                sg2 = small.tile([G, 2], F32R, name="sg2")
                nc.vector.tensor_copy(out=sg2[:, 0:1], in_=gss[:, 0:1])
                for it in range(3):
                    t1 = small.tile([G, 1], F32, name="nw_t1")
                    nc.vector.tensor_mul(out=t1[:], in0=r[:], in1=r[:])
                    t2 = small.tile([G, 1], F32, name="nw_t2")
                    nc.vector.scalar_tensor_tensor(
                        out=t2[:], in0=t1[:], scalar=-0.5, in1=gv[:],
                        op0=OP.mult, op1=OP.mult,
                    )
                    dst = sg2[:, 1:2] if it == 2 else small.tile(
                        [G, 1], F32, name="nw_r"
                    )
                    nc.vector.scalar_tensor_tensor(
                        out=dst, in0=t2[:], scalar=1.5, in1=r[:],
                        op0=OP.add, op1=OP.mult,
                    )
                    if it < 2:
                        r = dst
                s["sg2"] = sg2

            def gn_post(s):
                """Broadcast group stats to channels; per-channel affine
                A = rstd*gamma, B2 = mean*A - beta (h computed as x*A - B2)."""
                a_t = small.tile([128, CT], F32, name="a_vec")
                b2_t = small.tile([128, CT], F32, name="b2_vec")
                for ct in range(CT):
                    csp = ps_m.tile([128, 2], F32, name="csp", tag="mm512")
                    nc.tensor.matmul(
                        csp[:], gt_t[:, ct, :], s["sg2"][:], start=True, stop=True
                    )
                    nc.vector.tensor_mul(
                        out=a_t[:, ct : ct + 1], in0=csp[:, 1:2],
                        in1=vec_t[:, GAM, ct : ct + 1],
                    )
                    nc.vector.scalar_tensor_tensor(
                        out=b2_t[:, ct : ct + 1], in0=csp[:, 0:1],
                        scalar=a_t[:, ct : ct + 1], in1=vec_t[:, BET, ct : ct + 1],
                        op0=OP.mult, op1=OP.subtract,
                    )
                s["a"], s["b2"] = a_t, b2_t
                ht = big.tile([128, CT, N], F32R, name="hT")
                for ct in range(CT):
                    nc.vector.tensor_scalar(
                        ht[:, ct, :], s["x"][:, ct, :],
                        a_t[:, ct : ct + 1], b2_t[:, ct : ct + 1],
                        OP.mult, OP.subtract,
                    )
                s["h"] = ht

            def stage_proj(s):
                """q,k / folded-u (natural) and vT (transposed) projections."""
                ht = s["h"]

                if qk_bias:
                    qt = big.tile([128, CT, N], QK_DT, name="qT")
                    kt = big.tile([128, CT, N], QK_DT, name="kT")
                    pairs = ((qt, w_tiles["wq"], BQ), (kt, w_tiles["wk"], BK))
                else:
                    # u = wa^T... : s[m,n] = sum_c h[c,m] u[c,n]
                    ut = big.tile([128, CT, N], QK_DT, name="qT")
                    pairs = ((ut, w_tiles["wa"], None),)
                for dst, w_t, bias_idx in pairs:
                    for co in range(CT):
                        acc = ps_a.tile([128, N], F32, name="acc", tag="acc")
                        for mch in range(MCH):
                            msl = slice(mch * 512, (mch + 1) * 512)
                            for ci in range(CT):
                                nc.tensor.matmul(
                                    acc[:, msl],
                                    w_t[:, ci, co * 128 : (co + 1) * 128],
                                    ht[:, ci, msl],
                                    start=(ci == 0),
                                    stop=(ci == CT - 1),
                                )
                        nc.scalar.activation(
                            out=dst[:, co, :], in_=acc[:], func=AF.Identity,
                            bias=(0.0 if bias_idx is None
                                  else vec_t[:, bias_idx, co : co + 1]),
                            scale=1.0,
                        )
                if qk_bias:
                    s["q"], s["k"] = qt, kt
                else:
                    # sT[n,m] = sum_c u[c,n] h[c,m]: u is stationary, h moving
                    s["q"], s["k"] = ht, ut

                vts = []
                for nt in range(NT):
                    vp = ps_m.tile([128, C], F32, name="vp", tag="mm512")
                    for ci in range(CT):
                        nc.tensor.matmul(
                            vp[:],
                            ht[:, ci, nt * 128 : (nt + 1) * 128],
                            wv_t[:, ci, :],
                            start=(ci == 0),
                            stop=(ci == CT - 1),
                        )
                    vt = vtp.tile([128, C], PV_DT, name=f"vt{nt}")
                    nc.vector.tensor_add(out=vt[:], in0=vp[:], in1=bvb_t[:])
                    vts.append(vt)
                s["v"] = vts

            def stage_b(s, nxt):
                """scores^T -> exp -> pT ; row sums; next batch's gn chain
                interleaved so its serial DVE latency hides under PE work."""
                rs = ps_rs.tile([128, N], F32, name="rsp")
                pts = []
                for nt in range(NT):
                    stp = ps_a.tile([128, N], F32, name="stp", tag="acc")
                    for mch in range(MCH):
                        msl = slice(mch * 512, (mch + 1) * 512)
                        for ci in range(CT):
                            nc.tensor.matmul(
                                stp[:, msl],
                                s["k"][:, ci, nt * 128 : (nt + 1) * 128],
                                s["q"][:, ci, msl],
                                start=(ci == 0),
                                stop=(ci == CT - 1),
                            )
                    pt = ptp.tile([128, N], PV_DT, name=f"pt{nt}")
                    nc.scalar.activation(
                        out=pt[:], in_=stp[:], func=AF.Exp, bias=0.0, scale=SCALE
                    )
                    pts.append(pt)
                    for mch in range(MCH):
                        msl = slice(mch * 512, (mch + 1) * 512)
                        nc.tensor.matmul(
                            rs[:, msl], ones_t[:], pt[:, msl],
                            start=(nt == 0), stop=(nt == NT - 1),
                        )
                    if nt == 4 and nxt is not None:
                        gn_pre(nxt)
                    if nt == 6 and nxt is not None:
                        gn_post(nxt)
                s["p"] = pts
                s["rs"] = rs

            def stage_c(s):
                """1/rowsum; attend (+normalize); project (+bias+residual)."""
                rcp = misc.tile([128, N], F32, name="rcp")
                nc.vector.reciprocal_approx_fast(out=rcp[:], in_=s["rs"][:])

                ont = big.tile([128, CT, N], F32R, name="onT")
                for ct in range(CT):
                    for mch in range(MCH):
                        msl = slice(mch * 512, (mch + 1) * 512)
                        ap_ = ps_m.tile([128, 512], F32, name="attp", tag="mm512")
                        for nt in range(NT):
                            nc.tensor.matmul(
                                ap_[:],
                                s["v"][nt][:, ct * 128 : (ct + 1) * 128],
                                s["p"][nt][:, msl],
                                start=(nt == 0),
                                stop=(nt == NT - 1),
                            )
                        nc.vector.tensor_mul(
                            out=ont[:, ct, msl], in0=ap_[:], in1=rcp[:, msl]
                        )

                outf = big.tile([128, CT, N], F32, name="outf")
                for co in range(CT):
                    for mch in range(MCH):
                        msl = slice(mch * 512, (mch + 1) * 512)
                        pp = ps_m.tile([128, 512], F32, name="pp", tag="mm512")
                        for ci in range(CT):
                            nc.tensor.matmul(
                                pp[:],
                                wp_t[:, ci, co * 128 : (co + 1) * 128],
                                ont[:, ci, msl],
                                start=(ci == 0),
                                stop=(ci == CT - 1),
                            )
                        nc.vector.scalar_tensor_tensor(
                            out=outf[:, co, msl],
                            in0=pp[:],
                            scalar=vec_t[:, BP, co : co + 1],
                            in1=s["x"][:, co, msl],
                            op0=OP.add,
                            op1=OP.add,
                        )
                nc.sync.dma_start(
                    out=out_d[s["b"]].rearrange("(ct p) n -> p ct n", p=128),
                    in_=outf[:],
                )

            # ---------------- emission schedule ----------------
            gn_pre(states[0])
            gn_post(states[0])
            stage_proj(states[0])
            for b in range(B_LOC):
                nxt = None
                if b + 1 < B_LOC:
                    states[b + 1] = load(b + 1)
                    nxt = states[b + 1]
                stage_b(states[b], nxt)
                stage_c(states[b])
                states[b] = None
                if nxt is not None:
                    stage_proj(nxt)

    nc.finalize()
    return nc


_NC = {}


def _get_nc(qk_bias):
    if qk_bias not in _NC:
        _NC[qk_bias] = _build_nc(qk_bias=qk_bias)
    return _NC[qk_bias]


def _make_in_maps(inputs, qk_bias):
    x = np.asarray(inputs["x"], dtype=np.float32).reshape(B, C, N)
    g8 = np.zeros((C, G), np.float32)
    for c in range(C):
        g8[c, c // 8] = 0.125
    gt = np.zeros((G, C), np.float32)
    for c in range(C):
        gt[c // 8, c] = 1.0
    vecs = np.stack(
        [
            np.asarray(inputs["gamma"], np.float32),
            np.asarray(inputs["beta"], np.float32),
            np.asarray(inputs["bq"], np.float32),
            np.asarray(inputs["bk"], np.float32),
            np.asarray(inputs["bp"], np.float32),
        ]
    )

    shared = {
        "wvT": np.ascontiguousarray(np.asarray(inputs["wv"], np.float32).T),
        "wpT": np.ascontiguousarray(np.asarray(inputs["wp"], np.float32).T),
        "vecs": vecs,
        "bv": np.asarray(inputs["bv"], np.float32),
        "g8": g8,
        "gt": gt,
        "ones": np.ones((128,), np.float32),
    }
    if qk_bias:
        shared["wqT"] = np.ascontiguousarray(np.asarray(inputs["wq"], np.float32).T)
        shared["wkT"] = np.ascontiguousarray(np.asarray(inputs["wk"], np.float32).T)
    else:
        wa = np.asarray(inputs["wk"], np.float64).T @ np.asarray(
            inputs["wq"], np.float64
        )
        shared["waT"] = np.ascontiguousarray(wa.astype(np.float32))
    in_maps = []
    for i in range(N_CORES):
        m = dict(shared)
        m["x"] = np.ascontiguousarray(x[i * B_LOC : (i + 1) * B_LOC])
        in_maps.append(m)
    return in_maps


def _run(inputs, trace=False):
    from concourse.bass_utils import run_bass_kernel_spmd

    qk_bias = bool(
        np.any(np.asarray(inputs["bq"])) or np.any(np.asarray(inputs["bk"]))
    )
    nc = _get_nc(qk_bias)
    in_maps = _make_in_maps(inputs, qk_bias)
    res = run_bass_kernel_spmd(
        nc, in_maps, core_ids=list(range(N_CORES)), trace=trace
    )
    out = np.concatenate([r["out"] for r in res.results], axis=0)
    return out.reshape(B, C, 32, 32).astype(np.float32), res


def kernel(**inputs) -> np.ndarray:
    out, _ = _run(inputs, trace=False)
    return out
